# revision 7
# baseline (speedup 1.0000x reference)
"""AnchorBankCAA fused segment-mean/EMA/loss kernel for 8 TRN2 NeuronCores.

Strategy (data-parallel over B per the sharding hint):
  - each core gets B/8 = 65536 rows of mu_tilde (contiguous shard)
  - per 128-row tile: build a (128, 768) fp16 one-hot of seg = d*128+y via
    is_equal against an iota constant; 6 per-domain matmuls accumulate
    per-(domain,class) feature sums + counts into 6 PSUM banks; a 7th
    matmul with the domain one-hot accumulates per-domain sum-of-squares
  - AllReduce the tiny partials (~0.78 MB) across the 8 cores
  - every core runs the replicated EMA update + CAA/stats losses; core 0's
    outputs are returned
"""
import sys

sys.path.insert(0, "/opt/trn_rl_repo")

import numpy as np
from concourse import bacc, mybir
from concourse.alu_op_type import AluOpType
from concourse.tile import TileContext
from concourse.bass_utils import run_bass_kernel_spmd

C = 128          # classes
ND = 6           # domains
D = 256          # feat dim
B = 524288
NCORES = 8
BC = B // NCORES  # rows per core
P = 128
NT = BC // P     # tiles per core (512)
NSUP = 8         # tiles per DMA super-chunk (1 MiB)
MOM = 0.9
SEGW = D + 1     # 257: feature sums + count column
CCA = P * ND * SEGW          # seg partial floats
CCN = CCA + ND * D           # + dsq partial floats

f32 = mybir.dt.float32
f16 = mybir.dt.float16
RG = [list(range(NCORES))]

_compiled = None


def _build():
    nc = bacc.Bacc(num_devices=NCORES)

    mu = nc.dram_tensor("mu", (BC, D), f32, kind="ExternalInput")
    segT = nc.dram_tensor("segT", (P, NT), f32, kind="ExternalInput")
    dT = nc.dram_tensor("dT", (P, NT), f32, kind="ExternalInput")
    anchors = nc.dram_tensor("anchors", (ND, C, D), f32, kind="ExternalInput")
    dmeans = nc.dram_tensor("dmeans", (ND, D), f32, kind="ExternalInput")
    dvars = nc.dram_tensor("dvars", (ND, D), f32, kind="ExternalInput")

    o_anch = nc.dram_tensor("o_anch", (ND, C, D), f32, kind="ExternalOutput")
    o_means = nc.dram_tensor("o_means", (ND, D), f32, kind="ExternalOutput")
    o_vars = nc.dram_tensor("o_vars", (ND, D), f32, kind="ExternalOutput")
    o_loss = nc.dram_tensor("o_loss", (1, 1), f32, kind="ExternalOutput")

    dst_scr = nc.dram_tensor("dst_scr", (ND, SEGW), f32, kind="Internal")
    cc_in = nc.dram_tensor("cc_in", (CCN,), f32, kind="Internal")
    cc_out = nc.dram_tensor("cc_out", (CCN,), f32, kind="Internal",
                            addr_space="Shared")

    iota768_d = nc.inline_tensor(
        np.tile(np.arange(ND * C, dtype=np.float16), (P, 1)), "iota768")
    iota6_d = nc.inline_tensor(
        np.tile(np.arange(ND, dtype=np.float16), (P, 1)), "iota6")
    ident_d = nc.inline_tensor(np.eye(P, dtype=np.float32), "ident")
    offdiag_d = nc.inline_tensor(
        (1.0 - np.eye(C, dtype=np.float32)), "offdiag")

    with TileContext(nc) as tc:
        with (
            tc.tile_pool(name="singles", bufs=1) as sg,
            tc.tile_pool(name="mu", bufs=3) as mup,
            tc.tile_pool(name="work", bufs=4) as wp,
        ):
            iota768 = sg.tile([P, ND * C], f16)
            nc.sync.dma_start(iota768[:], iota768_d[:])
            iota6 = sg.tile([P, ND], f16)
            nc.sync.dma_start(iota6[:], iota6_d[:])
            segf = sg.tile([P, NT], f32)
            nc.sync.dma_start(segf[:], segT[:])
            dTf = sg.tile([P, NT], f32)
            nc.sync.dma_start(dTf[:], dT[:])

            accctx = tc.tile_pool(name="acc", bufs=1, space="PSUM")
            pacc = accctx.__enter__()
            seg_acc = [pacc.tile([P, SEGW], f32, tag=f"seg{d}",
                                 name=f"seg_acc{d}")
                       for d in range(ND)]
            dsq_acc = pacc.tile([ND, D], f32, tag="dsq")

            muv = mu.ap().rearrange("(s u p) f -> s p u f", p=P, u=NSUP)
            for s in range(NT // NSUP):
                sup = mup.tile([P, NSUP * D], f32)
                nc.sync.dma_start(
                    sup[:].rearrange("p (u f) -> p u f", u=NSUP), muv[s])
                for u in range(NSUP):
                    t = s * NSUP + u
                    msl = sup[:, u * D:(u + 1) * D]
                    muh = wp.tile([P, SEGW], f16, tag="muh")
                    nc.vector.tensor_copy(muh[:, :D], msl)
                    nc.gpsimd.memset(muh[:, D:SEGW], 1.0)
                    oh = wp.tile([P, ND * C], f16, tag="oh")
                    nc.vector.tensor_scalar(
                        oh[:], iota768[:], segf[:, t:t + 1], None,
                        AluOpType.is_equal)
                    dmask = wp.tile([P, ND], f16, tag="dm")
                    nc.vector.tensor_scalar(
                        dmask[:], iota6[:], dTf[:, t:t + 1], None,
                        AluOpType.is_equal)
                    musq = wp.tile([P, D], f16, tag="sq")
                    nc.scalar.square(musq[:], msl)
                    st, sp = (t == 0), (t == NT - 1)
                    for d in range(ND):
                        nc.tensor.matmul(
                            seg_acc[d][:], oh[:, d * C:(d + 1) * C], muh[:],
                            start=st, stop=sp)
                    nc.tensor.matmul(dsq_acc[:], dmask[:], musq[:],
                                     start=st, stop=sp)

            # ---- pack partials and AllReduce ----
            ccb = sg.tile([P, ND * SEGW], f32)
            for d in range(ND):
                nc.vector.tensor_copy(
                    ccb[:, d * SEGW:(d + 1) * SEGW], seg_acc[d][:])
            dsqb = sg.tile([ND, D], f32)
            nc.vector.tensor_copy(dsqb[:], dsq_acc[:])
            accctx.__exit__(None, None, None)
            pfinctx = tc.tile_pool(name="pfin", bufs=4, space="PSUM")
            pfin = pfinctx.__enter__()

            cc_in_a = cc_in.ap()[0:CCA].rearrange("(p j) -> p j", p=P)
            cc_in_b = cc_in.ap()[CCA:CCN].rearrange("(p j) -> p j", p=ND)
            nc.sync.dma_start(cc_in_a, ccb[:])
            nc.sync.dma_start(cc_in_b, dsqb[:])
            nc.gpsimd.collective_compute(
                "AllReduce", AluOpType.add, replica_groups=RG,
                ins=[cc_in.ap()], outs=[cc_out.ap()])
            segg = sg.tile([P, ND * SEGW], f32)
            nc.sync.dma_start(
                segg[:], cc_out.ap()[0:CCA].rearrange("(p j) -> p j", p=P))
            dsqg = sg.tile([ND, D], f32)
            nc.sync.dma_start(
                dsqg[:], cc_out.ap()[CCA:CCN].rearrange("(p j) -> p j", p=ND))

            # ---- replicated final phase ----
            ident = sg.tile([P, P], f32)
            nc.sync.dma_start(ident[:], ident_d[:])
            offdiag = sg.tile([C, C], f32)
            nc.sync.dma_start(offdiag[:], offdiag_d[:])
            anch = sg.tile([P, ND * D], f32)
            nc.sync.dma_start(
                anch[:].rearrange("c (a f) -> c a f", a=ND),
                anchors.ap().rearrange("a c f -> c a f"))
            dmns = sg.tile([ND, D], f32)
            nc.sync.dma_start(dmns[:], dmeans.ap())
            dvrs = sg.tile([ND, D], f32)
            nc.sync.dma_start(dvrs[:], dvars.ap())

            ones128 = sg.tile([P, 1], f32)
            nc.vector.memset(ones128[:], 1.0)
            ones6 = sg.tile([ND, 1], f32)
            nc.vector.memset(ones6[:], 1.0)
            onesrow = sg.tile([1, C], f32)
            nc.vector.memset(onesrow[:], 1.0)

            # new anchors: A + 0.1*(cnt>0)*(seg_mean - A)
            newA = sg.tile([P, ND * D], f32)
            for d in range(ND):
                seg_d = segg[:, d * SEGW:d * SEGW + D]
                cnt_d = segg[:, d * SEGW + D:d * SEGW + D + 1]
                cntc = wp.tile([P, 1], f32, tag="f1")
                nc.vector.tensor_scalar(cntc[:], cnt_d, 1.0, None,
                                        AluOpType.max)
                rec = wp.tile([P, 1], f32, tag="f2")
                nc.vector.reciprocal(rec[:], cntc[:])
                mean_d = wp.tile([P, D], f32, tag="fD")
                nc.vector.tensor_scalar(mean_d[:], seg_d, rec[:, 0:1], None,
                                        AluOpType.mult)
                has01 = wp.tile([P, 1], f32, tag="f3")
                nc.vector.tensor_scalar(has01[:], cnt_d, 0.0, 1.0 - MOM,
                                        AluOpType.is_gt, AluOpType.mult)
                a_d = anch[:, d * D:(d + 1) * D]
                diff = wp.tile([P, D], f32, tag="fD2")
                nc.vector.tensor_tensor(diff[:], mean_d[:], a_d,
                                        AluOpType.subtract)
                nc.vector.tensor_scalar(diff[:], diff[:], has01[:, 0:1], None,
                                        AluOpType.mult)
                nc.vector.tensor_tensor(newA[:, d * D:(d + 1) * D], a_d,
                                        diff[:], AluOpType.add)
            nc.sync.dma_start(
                o_anch.ap().rearrange("a c f -> c a f"),
                newA[:].rearrange("c (a f) -> c a f", a=ND))

            # class mean over domains (= A_mean)
            cm = sg.tile([P, D], f32)
            nc.vector.tensor_tensor(cm[:], newA[:, 0:D], newA[:, D:2 * D],
                                    AluOpType.add)
            for d in range(2, ND):
                nc.vector.tensor_tensor(cm[:], cm[:],
                                        newA[:, d * D:(d + 1) * D],
                                        AluOpType.add)
            nc.vector.tensor_scalar(cm[:], cm[:], 1.0 / ND, None,
                                    AluOpType.mult)

            # loss_intra = mean((newA - cm)^2)
            li = sg.tile([P, 1], f32)
            sqs = wp.tile([P, D], f32, tag="fD")
            acc_d = wp.tile([P, 1], f32, tag="f1")
            for d in range(ND):
                dif = wp.tile([P, D], f32, tag="fD2")
                nc.vector.tensor_tensor(dif[:], newA[:, d * D:(d + 1) * D],
                                        cm[:], AluOpType.subtract)
                ac = wp.tile([P, 1], f32, tag=f"li{d}")
                nc.scalar.activation(sqs[:], dif[:],
                                     mybir.ActivationFunctionType.Square,
                                     accum_out=ac[:])
                if d == 0:
                    nc.vector.tensor_copy(li[:], ac[:])
                else:
                    nc.vector.tensor_tensor(li[:], li[:], ac[:],
                                            AluOpType.add)
            nc.vector.tensor_scalar(li[:], li[:], 1.0 / (ND * C * D), None,
                                    AluOpType.mult)

            # loss_inter: pairwise distances of cm rows
            sqp = sg.tile([P, 1], f32)   # row sums of cm^2
            cm2 = wp.tile([P, D], f32, tag="fD")
            nc.scalar.activation(cm2[:], cm[:],
                                 mybir.ActivationFunctionType.Square,
                                 accum_out=sqp[:])
            amt = sg.tile([P, D], f32)      # cm transposed (2 blocks)
            amtn = sg.tile([P, D], f32)     # -2 * cm^T
            for k in range(2):
                trp = pfin.tile([P, P], f32, tag="fp")
                nc.tensor.transpose(trp[:], cm[:, k * P:(k + 1) * P],
                                    ident[:])
                nc.vector.tensor_copy(amt[:, k * P:(k + 1) * P], trp[:])
                nc.vector.tensor_scalar(amtn[:, k * P:(k + 1) * P], trp[:],
                                        -2.0, None, AluOpType.mult)
            sqrp = pfin.tile([1, P], f32, tag="fp")
            nc.tensor.transpose(sqrp[:], sqp[:], ident[:])
            sqr = sg.tile([1, C], f32)
            nc.vector.tensor_copy(sqr[:], sqrp[:])

            d2p = pfin.tile([P, C], f32, tag="fp")
            nc.tensor.matmul(d2p[:], amt[:, 0:P], amtn[:, 0:P],
                             start=True, stop=False)
            nc.tensor.matmul(d2p[:], amt[:, P:2 * P], amtn[:, P:2 * P],
                             start=False, stop=False)
            nc.tensor.matmul(d2p[:], onesrow[:], sqr[:],
                             start=False, stop=False)
            nc.tensor.matmul(d2p[:], sqr[:], onesrow[:],
                             start=False, stop=True)
            d2s = sg.tile([P, C], f32)
            nc.vector.tensor_scalar(d2s[:], d2p[:], 1e-12, None,
                                    AluOpType.max)
            dst = wp.tile([P, C], f32, tag="fD")
            nc.scalar.activation(dst[:], d2s[:],
                                 mybir.ActivationFunctionType.Sqrt)
            rel = wp.tile([P, C], f32, tag="fD2")
            nc.scalar.activation(rel[:], dst[:],
                                 mybir.ActivationFunctionType.Relu,
                                 bias=1.0, scale=-1.0)
            nc.vector.tensor_tensor(rel[:], rel[:], offdiag[:],
                                    AluOpType.mult)
            ri = sg.tile([P, 1], f32)
            nc.vector.reduce_sum(ri[:], rel[:], axis=mybir.AxisListType.X)
            nc.vector.tensor_scalar(ri[:], ri[:], 1.0 / (C * (C - 1)), None,
                                    AluOpType.mult)

            # per-domain stats: sum over classes (partition axis); matmul
            # outputs land on partition 0, bounce via DRAM to stack as (6,.)
            for d in range(ND):
                pt = pfin.tile([1, SEGW], f32, tag="fp")
                nc.tensor.matmul(pt[:], ones128[:],
                                 segg[:, d * SEGW:(d + 1) * SEGW],
                                 start=True, stop=True)
                row = wp.tile([1, SEGW], f32, tag="dstrow")
                nc.vector.tensor_copy(row[:], pt[:])
                nc.sync.dma_start(dst_scr.ap()[d:d + 1, :], row[:])
            dsts = sg.tile([ND, SEGW], f32)
            nc.sync.dma_start(dsts[:], dst_scr.ap())
            d_sum = dsts[:, 0:D]
            d_cnt = dsts[:, D:SEGW]

            safe = sg.tile([ND, 1], f32)
            nc.vector.tensor_scalar(safe[:], d_cnt, 1.0, None, AluOpType.max)
            rec6 = sg.tile([ND, 1], f32)
            nc.vector.reciprocal(rec6[:], safe[:])
            b_mean = sg.tile([ND, D], f32)
            nc.vector.tensor_scalar(b_mean[:], d_sum, rec6[:, 0:1], None,
                                    AluOpType.mult)
            bm2 = wp.tile([ND, D], f32, tag="g1")
            nc.scalar.activation(bm2[:], b_mean[:],
                                 mybir.ActivationFunctionType.Square)
            nc.vector.tensor_scalar(bm2[:], bm2[:], safe[:, 0:1], None,
                                    AluOpType.mult)
            b_var = sg.tile([ND, D], f32)
            nc.vector.tensor_tensor(b_var[:], dsqg[:], bm2[:],
                                    AluOpType.subtract)
            cm1 = sg.tile([ND, 1], f32)
            nc.vector.tensor_scalar(cm1[:], d_cnt, -1.0, 1.0,
                                    AluOpType.add, AluOpType.max)
            recd = sg.tile([ND, 1], f32)
            nc.vector.reciprocal(recd[:], cm1[:])
            nc.vector.tensor_scalar(b_var[:], b_var[:], recd[:, 0:1], None,
                                    AluOpType.mult)
            g01 = sg.tile([ND, 1], f32)
            nc.vector.tensor_scalar(g01[:], d_cnt, 1.0, 1.0 - MOM,
                                    AluOpType.is_gt, AluOpType.mult)

            newM = sg.tile([ND, D], f32)
            nc.vector.tensor_tensor(newM[:], b_mean[:], dmns[:],
                                    AluOpType.subtract)
            nc.vector.tensor_scalar(newM[:], newM[:], g01[:, 0:1], None,
                                    AluOpType.mult)
            nc.vector.tensor_tensor(newM[:], dmns[:], newM[:], AluOpType.add)
            nc.sync.dma_start(o_means.ap(), newM[:])
            newV = sg.tile([ND, D], f32)
            nc.vector.tensor_tensor(newV[:], b_var[:], dvrs[:],
                                    AluOpType.subtract)
            nc.vector.tensor_scalar(newV[:], newV[:], g01[:, 0:1], None,
                                    AluOpType.mult)
            nc.vector.tensor_tensor(newV[:], dvrs[:], newV[:], AluOpType.add)
            nc.sync.dma_start(o_vars.ap(), newV[:])

            # global mean/var of the updated stats
            def _colmean6(src_ap, nm):
                pt = pfin.tile([1, D], f32, tag="fp", name=f"pt_{nm}")
                nc.tensor.matmul(pt[:], ones6[:], src_ap, start=True,
                                 stop=True)
                out = sg.tile([1, D], f32, tag=nm, name=nm)
                nc.vector.tensor_scalar(out[:], pt[:], 1.0 / ND, None,
                                        AluOpType.mult)
                return out

            gm = _colmean6(newM[:], "gm")
            gv = _colmean6(newV[:], "gv")

            # loss_mean / loss_var via E[x^2] - gm^2 identity
            def _spread_loss(x_ap, g_ap, nm):
                x2 = wp.tile([ND, D], f32, tag="g1", name=f"x2_{nm}")
                nc.scalar.activation(x2[:], x_ap,
                                     mybir.ActivationFunctionType.Square)
                m2 = _colmean6(x2[:], f"m2_{nm}")
                g2 = wp.tile([1, D], f32, tag="g2", name=f"g2_{nm}")
                nc.scalar.activation(g2[:], g_ap,
                                     mybir.ActivationFunctionType.Square)
                df = wp.tile([1, D], f32, tag="g4", name=f"df_{nm}")
                nc.vector.tensor_tensor(df[:], m2[:], g2[:],
                                        AluOpType.subtract)
                out = sg.tile([1, 1], f32, tag=nm, name=nm)
                nc.vector.reduce_sum(out[:], df[:], axis=mybir.AxisListType.X)
                nc.vector.tensor_scalar(out[:], out[:], 1.0 / D, None,
                                        AluOpType.mult)
                return out

            l_mean = _spread_loss(newM[:], gm[:], "lmean")
            l_var = _spread_loss(newV[:], gv[:], "lvar")

            # mu_mean / mu_var from global sums
            mmp = pfin.tile([1, SEGW], f32, tag="fp")
            nc.tensor.matmul(mmp[:], ones6[:], dsts[:], start=True, stop=True)
            mu_mean = sg.tile([1, D], f32)
            nc.vector.tensor_scalar(mu_mean[:], mmp[:, 0:D], 1.0 / B, None,
                                    AluOpType.mult)
            msp = pfin.tile([1, D], f32, tag="fp")
            nc.tensor.matmul(msp[:], ones6[:], dsqg[:], start=True, stop=True)
            mu_sq = sg.tile([1, D], f32)
            nc.vector.tensor_scalar(mu_sq[:], msp[:], 1.0 / B, None,
                                    AluOpType.mult)
            mm2 = wp.tile([1, D], f32, tag="g2")
            nc.scalar.activation(mm2[:], mu_mean[:],
                                 mybir.ActivationFunctionType.Square)
            mu_var = sg.tile([1, D], f32)
            nc.vector.tensor_tensor(mu_var[:], mu_sq[:], mm2[:],
                                    AluOpType.subtract)

            def _mse_row(a_ap, b_ap, nm):
                df = wp.tile([1, D], f32, tag="g2", name=f"df_{nm}")
                nc.vector.tensor_tensor(df[:], a_ap, b_ap,
                                        AluOpType.subtract)
                s2 = wp.tile([1, D], f32, tag="g3", name=f"s2_{nm}")
                out = sg.tile([1, 1], f32, tag=nm, name=nm)
                nc.scalar.activation(s2[:], df[:],
                                     mybir.ActivationFunctionType.Square,
                                     accum_out=out[:])
                nc.vector.tensor_scalar(out[:], out[:], 1.0 / D, None,
                                        AluOpType.mult)
                return out

            l_mu_mean = _mse_row(mu_mean[:], gm[:], "lmumean")
            l_mu_var = _mse_row(mu_var[:], gv[:], "lmuvar")

            # total loss: accumulate all pieces into one PSUM scalar
            lossp = pfin.tile([1, 1], f32, tag="fp")
            nc.tensor.matmul(lossp[:], ones128[:], li[:],
                             start=True, stop=False)
            nc.tensor.matmul(lossp[:], ones128[:], ri[:],
                             start=False, stop=False)
            one1 = sg.tile([1, 1], f32)
            nc.vector.memset(one1[:], 1.0)
            pieces = [l_mean, l_var, l_mu_mean, l_mu_var]
            for i, pc in enumerate(pieces):
                nc.tensor.matmul(lossp[:], one1[:], pc[:],
                                 start=False, stop=(i == len(pieces) - 1))
            lout = sg.tile([1, 1], f32)
            nc.vector.tensor_copy(lout[:], lossp[:])
            nc.sync.dma_start(o_loss.ap(), lout[:])
            pfinctx.__exit__(None, None, None)

    nc.compile()
    return nc


def _prep_inputs(mu_tilde, anchors, domain_means, domain_vars, y_true,
                 d_true):
    mu_tilde = np.ascontiguousarray(np.asarray(mu_tilde, dtype=np.float32))
    anchors = np.ascontiguousarray(np.asarray(anchors, dtype=np.float32))
    domain_means = np.ascontiguousarray(
        np.asarray(domain_means, dtype=np.float32))
    domain_vars = np.ascontiguousarray(
        np.asarray(domain_vars, dtype=np.float32))
    y = np.asarray(y_true).astype(np.int32)
    d = np.asarray(d_true).astype(np.int32)
    seg = (d * C + y).astype(np.float32)
    df16 = d.astype(np.float32)
    in_maps = []
    for i in range(NCORES):
        lo, hi = i * BC, (i + 1) * BC
        segT = np.ascontiguousarray(seg[lo:hi].reshape(NT, P).T)
        dT = np.ascontiguousarray(df16[lo:hi].reshape(NT, P).T)
        in_maps.append({
            "mu": mu_tilde[lo:hi],
            "segT": segT,
            "dT": dT,
            "anchors": anchors,
            "dmeans": domain_means,
            "dvars": domain_vars,
        })
    return in_maps


def get_compiled():
    global _compiled
    if _compiled is None:
        _compiled = _build()
    return _compiled


def run(in_maps, **kw):
    nc = get_compiled()
    return run_bass_kernel_spmd(nc, in_maps, core_ids=list(range(NCORES)),
                                **kw)


def kernel(mu_tilde, anchors, domain_means, domain_vars, y_true, d_true):
    in_maps = _prep_inputs(mu_tilde, anchors, domain_means, domain_vars,
                           y_true, d_true)
    res = run(in_maps)
    r0 = res.results[0]
    return (
        r0["o_anch"].astype(np.float32),
        r0["o_means"].astype(np.float32),
        r0["o_vars"].astype(np.float32),
        np.float32(r0["o_loss"].reshape(())),
    )


# revision 8
# speedup vs baseline: 1.3593x; 1.3593x over previous
"""AnchorBankCAA fused segment-mean/EMA/loss kernel for 8 TRN2 NeuronCores.

Strategy (data-parallel over B, rows domain-sorted host-side):
  - host sorts rows by domain and packs them into single-domain groups of
    3072 rows (24 tiles of 128), padded with inert rows (mu=0, y=999);
    22 groups per core (67584 rows, +3.1% padding)
  - mu ships as fp16 with per-tile layout [mu | mu^2-slot]; ACT/GpSimd
    alternate computing the squares into the slot
  - per tile: ONE matmul — class one-hot (is_equal vs iota) as stationary,
    [mu | mu^2] (128, 512) moving — accumulating [feature sums | sqsums]
    per class into a ping-pong PSUM stage bank
  - per group: 6 masked-identity matmuls flush the stage into 6 per-domain
    PSUM accumulators (mask = host-provided group-domain one-hot)
  - AllReduce the (128, 6*512) f32 partials, then a replicated final phase
    (EMA + CAA/stats losses) computes the outputs; counts come from a host
    bincount (index metadata only)
"""
import sys

sys.path.insert(0, "/opt/trn_rl_repo")

import numpy as np
from concourse import bacc, mybir
from concourse.alu_op_type import AluOpType
from concourse.tile import TileContext
from concourse.bass_utils import run_bass_kernel_spmd

C = 128          # classes
ND = 6           # domains
D = 256          # feat dim
B = 524288
NCORES = 8
P = 128
GT = 24          # tiles per group
GR = GT * P      # rows per group (3072)
NG = 22          # groups per core
NTp = NG * GT    # tiles per core (528)
R = NTp * P      # padded rows per core (67584)
MOM = 0.9
W = 2 * D        # 512: [sums | sqsums] stage width
CCN = P * ND * W  # AllReduce payload floats (128*6*512)

f32 = mybir.dt.float32
f16 = mybir.dt.float16
RG = [list(range(NCORES))]

_compiled = None


def _build():
    nc = bacc.Bacc(num_devices=NCORES)

    mu = nc.dram_tensor("mu", (R, D), f16, kind="ExternalInput")
    yT = nc.dram_tensor("yT", (P, NTp), f32, kind="ExternalInput")
    dgo = nc.dram_tensor("dgo", (P, NG * ND), f32, kind="ExternalInput")
    cnts = nc.dram_tensor("cnts", (P, ND), f32, kind="ExternalInput")
    dcnt = nc.dram_tensor("dcnt", (ND, 1), f32, kind="ExternalInput")
    anchors = nc.dram_tensor("anchors", (ND, C, D), f32, kind="ExternalInput")
    dmeans = nc.dram_tensor("dmeans", (ND, D), f32, kind="ExternalInput")
    dvars = nc.dram_tensor("dvars", (ND, D), f32, kind="ExternalInput")

    o_anch = nc.dram_tensor("o_anch", (ND, C, D), f32, kind="ExternalOutput")
    o_means = nc.dram_tensor("o_means", (ND, D), f32, kind="ExternalOutput")
    o_vars = nc.dram_tensor("o_vars", (ND, D), f32, kind="ExternalOutput")
    o_loss = nc.dram_tensor("o_loss", (1, 1), f32, kind="ExternalOutput")

    dst_scr = nc.dram_tensor("dst_scr", (ND, W), f32, kind="Internal")
    cc_in = nc.dram_tensor("cc_in", (CCN,), f32, kind="Internal")
    cc_out = nc.dram_tensor("cc_out", (CCN,), f32, kind="Internal",
                            addr_space="Shared")

    iota128_d = nc.inline_tensor(
        np.tile(np.arange(C, dtype=np.float16), (P, 1)), "iota128")
    ident16_d = nc.inline_tensor(np.eye(P, dtype=np.float16), "ident16")
    ident_d = nc.inline_tensor(np.eye(P, dtype=np.float32), "ident")
    offdiag_d = nc.inline_tensor(
        (1.0 - np.eye(C, dtype=np.float32)), "offdiag")

    with TileContext(nc) as tc:
        with (
            tc.tile_pool(name="singles", bufs=1) as sg,
            tc.tile_pool(name="grp", bufs=2) as grp,
            tc.tile_pool(name="work", bufs=4) as wp,
        ):
            iota128 = sg.tile([P, C], f16)
            nc.sync.dma_start(iota128[:], iota128_d[:])
            ident16 = sg.tile([P, P], f16)
            nc.sync.dma_start(ident16[:], ident16_d[:])
            yTs = sg.tile([P, NTp], f32)
            nc.sync.dma_start(yTs[:], yT[:])
            dgos = sg.tile([P, NG * ND], f32)
            nc.sync.dma_start(dgos[:], dgo[:])

            accctx = tc.tile_pool(name="acc", bufs=1, space="PSUM")
            pacc = accctx.__enter__()
            stage = [pacc.tile([P, W], f32, tag=f"stage{k}",
                               name=f"stage{k}") for k in range(2)]
            finals = [pacc.tile([P, W], f32, tag=f"fin{d}",
                                name=f"fin{d}") for d in range(ND)]

            # dram view: group g, tile-in-group u, partition p, feat f
            muv = mu.ap().rearrange("(g u p) f -> g p u f", u=GT, p=P)
            for g in range(NG):
                gt = grp.tile([P, GT * W], f16, name="gt", tag="gt")
                # mu -> first half of each tile slot
                nc.sync.dma_start(
                    gt[:].rearrange("p (u w) -> p u w", u=GT)[:, :, 0:D],
                    muv[g])
                stg = stage[g % 2]
                for u in range(GT):
                    t = g * GT + u
                    msl = gt[:, u * W:u * W + D]
                    sqs = gt[:, u * W + D:(u + 1) * W]
                    # squares alternate ACT / GpSimd
                    if u % 2 == 0:
                        nc.scalar.square(sqs, msl)
                    else:
                        nc.gpsimd.tensor_tensor(sqs, msl, msl,
                                                AluOpType.mult)
                    oh = wp.tile([P, C], f16, tag="oh", name="oh")
                    nc.vector.tensor_scalar(
                        oh[:], iota128[:], yTs[:, t:t + 1], None,
                        AluOpType.is_equal)
                    nc.tensor.matmul(stg[:], oh[:], gt[:, u * W:(u + 1) * W],
                                     start=(u == 0), stop=(u == GT - 1))
                # flush stage into per-domain finals via masked identities
                stgs = wp.tile([P, W], f16, tag="stgs", name="stgs")
                nc.vector.tensor_copy(stgs[:], stg[:])
                for d in range(ND):
                    mid = wp.tile([P, P], f16, tag=f"mid{d}",
                                  name=f"mid{d}")
                    nc.vector.tensor_scalar(
                        mid[:], ident16[:], dgos[:, g * ND + d:g * ND + d + 1],
                        None, AluOpType.mult)
                    nc.tensor.matmul(finals[d][:], mid[:], stgs[:],
                                     start=(g == 0), stop=(g == NG - 1))

            # ---- pack partials and AllReduce ----
            ccb = sg.tile([P, ND * W], f32)
            for d in range(ND):
                nc.vector.tensor_copy(ccb[:, d * W:(d + 1) * W],
                                      finals[d][:])
            accctx.__exit__(None, None, None)
            pfinctx = tc.tile_pool(name="pfin", bufs=4, space="PSUM")
            pfin = pfinctx.__enter__()

            nc.sync.dma_start(
                cc_in.ap().rearrange("(p j) -> p j", p=P), ccb[:])
            nc.gpsimd.collective_compute(
                "AllReduce", AluOpType.add, replica_groups=RG,
                ins=[cc_in.ap()], outs=[cc_out.ap()])
            segg = sg.tile([P, ND * W], f32)
            nc.sync.dma_start(
                segg[:], cc_out.ap().rearrange("(p j) -> p j", p=P))

            # ---- replicated final phase ----
            ident = sg.tile([P, P], f32)
            nc.sync.dma_start(ident[:], ident_d[:])
            offdiag = sg.tile([C, C], f32)
            nc.sync.dma_start(offdiag[:], offdiag_d[:])
            anch = sg.tile([P, ND * D], f32)
            nc.sync.dma_start(
                anch[:].rearrange("c (a f) -> c a f", a=ND),
                anchors.ap().rearrange("a c f -> c a f"))
            dmns = sg.tile([ND, D], f32)
            nc.sync.dma_start(dmns[:], dmeans.ap())
            dvrs = sg.tile([ND, D], f32)
            nc.sync.dma_start(dvrs[:], dvars.ap())
            cnts_s = sg.tile([P, ND], f32)
            nc.sync.dma_start(cnts_s[:], cnts.ap())
            dcnt_s = sg.tile([ND, 1], f32)
            nc.sync.dma_start(dcnt_s[:], dcnt.ap())

            ones128 = sg.tile([P, 1], f32)
            nc.vector.memset(ones128[:], 1.0)
            ones6 = sg.tile([ND, 1], f32)
            nc.vector.memset(ones6[:], 1.0)
            onesrow = sg.tile([1, C], f32)
            nc.vector.memset(onesrow[:], 1.0)

            # new anchors: A + 0.1*(cnt>0)*(seg_mean - A)
            newA = sg.tile([P, ND * D], f32)
            for d in range(ND):
                seg_d = segg[:, d * W:d * W + D]
                cnt_d = cnts_s[:, d:d + 1]
                cntc = wp.tile([P, 1], f32, tag="f1", name=f"cntc{d}")
                nc.vector.tensor_scalar(cntc[:], cnt_d, 1.0, None,
                                        AluOpType.max)
                rec = wp.tile([P, 1], f32, tag="f2", name=f"rec{d}")
                nc.vector.reciprocal(rec[:], cntc[:])
                mean_d = wp.tile([P, D], f32, tag="fD", name=f"mean{d}")
                nc.vector.tensor_scalar(mean_d[:], seg_d, rec[:, 0:1], None,
                                        AluOpType.mult)
                has01 = wp.tile([P, 1], f32, tag="f3", name=f"has{d}")
                nc.vector.tensor_scalar(has01[:], cnt_d, 0.0, 1.0 - MOM,
                                        AluOpType.is_gt, AluOpType.mult)
                a_d = anch[:, d * D:(d + 1) * D]
                diff = wp.tile([P, D], f32, tag="fD2", name=f"diff{d}")
                nc.vector.tensor_tensor(diff[:], mean_d[:], a_d,
                                        AluOpType.subtract)
                nc.vector.tensor_scalar(diff[:], diff[:], has01[:, 0:1], None,
                                        AluOpType.mult)
                nc.vector.tensor_tensor(newA[:, d * D:(d + 1) * D], a_d,
                                        diff[:], AluOpType.add)
            nc.sync.dma_start(
                o_anch.ap().rearrange("a c f -> c a f"),
                newA[:].rearrange("c (a f) -> c a f", a=ND))

            # class mean over domains (= A_mean)
            cm = sg.tile([P, D], f32)
            nc.vector.tensor_tensor(cm[:], newA[:, 0:D], newA[:, D:2 * D],
                                    AluOpType.add)
            for d in range(2, ND):
                nc.vector.tensor_tensor(cm[:], cm[:],
                                        newA[:, d * D:(d + 1) * D],
                                        AluOpType.add)
            nc.vector.tensor_scalar(cm[:], cm[:], 1.0 / ND, None,
                                    AluOpType.mult)

            # loss_intra = mean((newA - cm)^2)
            li = sg.tile([P, 1], f32)
            sqscr = wp.tile([P, D], f32, tag="fD", name="sqscr")
            for d in range(ND):
                dif = wp.tile([P, D], f32, tag="fD2", name=f"dif{d}")
                nc.vector.tensor_tensor(dif[:], newA[:, d * D:(d + 1) * D],
                                        cm[:], AluOpType.subtract)
                ac = wp.tile([P, 1], f32, tag=f"li{d}", name=f"liac{d}")
                nc.scalar.activation(sqscr[:], dif[:],
                                     mybir.ActivationFunctionType.Square,
                                     accum_out=ac[:])
                if d == 0:
                    nc.vector.tensor_copy(li[:], ac[:])
                else:
                    nc.vector.tensor_tensor(li[:], li[:], ac[:],
                                            AluOpType.add)
            nc.vector.tensor_scalar(li[:], li[:], 1.0 / (ND * C * D), None,
                                    AluOpType.mult)

            # loss_inter: pairwise distances of cm rows
            sqp = sg.tile([P, 1], f32)
            cm2 = wp.tile([P, D], f32, tag="fD", name="cm2")
            nc.scalar.activation(cm2[:], cm[:],
                                 mybir.ActivationFunctionType.Square,
                                 accum_out=sqp[:])
            amt = sg.tile([P, D], f32)
            amtn = sg.tile([P, D], f32)
            for k in range(2):
                trp = pfin.tile([P, P], f32, tag="fp", name=f"trp{k}")
                nc.tensor.transpose(trp[:], cm[:, k * P:(k + 1) * P],
                                    ident[:])
                nc.vector.tensor_copy(amt[:, k * P:(k + 1) * P], trp[:])
                nc.vector.tensor_scalar(amtn[:, k * P:(k + 1) * P], trp[:],
                                        -2.0, None, AluOpType.mult)
            sqrp = pfin.tile([1, P], f32, tag="fp", name="sqrp")
            nc.tensor.transpose(sqrp[:], sqp[:], ident[:])
            sqr = sg.tile([1, C], f32)
            nc.vector.tensor_copy(sqr[:], sqrp[:])

            d2p = pfin.tile([P, C], f32, tag="fp", name="d2p")
            nc.tensor.matmul(d2p[:], amt[:, 0:P], amtn[:, 0:P],
                             start=True, stop=False)
            nc.tensor.matmul(d2p[:], amt[:, P:2 * P], amtn[:, P:2 * P],
                             start=False, stop=False)
            nc.tensor.matmul(d2p[:], onesrow[:], sqr[:],
                             start=False, stop=False)
            nc.tensor.matmul(d2p[:], sqr[:], onesrow[:],
                             start=False, stop=True)
            d2s = sg.tile([P, C], f32)
            nc.vector.tensor_scalar(d2s[:], d2p[:], 1e-12, None,
                                    AluOpType.max)
            dst = wp.tile([P, C], f32, tag="fD", name="dst")
            nc.scalar.activation(dst[:], d2s[:],
                                 mybir.ActivationFunctionType.Sqrt)
            rel = wp.tile([P, C], f32, tag="fD2", name="rel")
            nc.scalar.activation(rel[:], dst[:],
                                 mybir.ActivationFunctionType.Relu,
                                 bias=1.0, scale=-1.0)
            nc.vector.tensor_tensor(rel[:], rel[:], offdiag[:],
                                    AluOpType.mult)
            ri = sg.tile([P, 1], f32)
            nc.vector.reduce_sum(ri[:], rel[:], axis=mybir.AxisListType.X)
            nc.vector.tensor_scalar(ri[:], ri[:], 1.0 / (C * (C - 1)), None,
                                    AluOpType.mult)

            # per-domain stats: [d_sum | d_sq] = column sums over classes
            for d in range(ND):
                pt = pfin.tile([1, W], f32, tag="fp", name=f"pt{d}")
                nc.tensor.matmul(pt[:], ones128[:],
                                 segg[:, d * W:(d + 1) * W],
                                 start=True, stop=True)
                row = wp.tile([1, W], f32, tag="dstrow", name=f"row{d}")
                nc.vector.tensor_copy(row[:], pt[:])
                nc.sync.dma_start(dst_scr.ap()[d:d + 1, :], row[:])
            dsts = sg.tile([ND, W], f32)
            nc.sync.dma_start(dsts[:], dst_scr.ap())
            d_sum = dsts[:, 0:D]
            d_sq = dsts[:, D:W]

            safe = sg.tile([ND, 1], f32)
            nc.vector.tensor_scalar(safe[:], dcnt_s[:], 1.0, None,
                                    AluOpType.max)
            rec6 = sg.tile([ND, 1], f32)
            nc.vector.reciprocal(rec6[:], safe[:])
            b_mean = sg.tile([ND, D], f32)
            nc.vector.tensor_scalar(b_mean[:], d_sum, rec6[:, 0:1], None,
                                    AluOpType.mult)
            bm2 = wp.tile([ND, D], f32, tag="g1", name="bm2")
            nc.scalar.activation(bm2[:], b_mean[:],
                                 mybir.ActivationFunctionType.Square)
            nc.vector.tensor_scalar(bm2[:], bm2[:], safe[:, 0:1], None,
                                    AluOpType.mult)
            b_var = sg.tile([ND, D], f32)
            nc.vector.tensor_tensor(b_var[:], d_sq, bm2[:],
                                    AluOpType.subtract)
            cm1 = sg.tile([ND, 1], f32)
            nc.vector.tensor_scalar(cm1[:], dcnt_s[:], -1.0, 1.0,
                                    AluOpType.add, AluOpType.max)
            recd = sg.tile([ND, 1], f32)
            nc.vector.reciprocal(recd[:], cm1[:])
            nc.vector.tensor_scalar(b_var[:], b_var[:], recd[:, 0:1], None,
                                    AluOpType.mult)
            g01 = sg.tile([ND, 1], f32)
            nc.vector.tensor_scalar(g01[:], dcnt_s[:], 1.0, 1.0 - MOM,
                                    AluOpType.is_gt, AluOpType.mult)

            newM = sg.tile([ND, D], f32)
            nc.vector.tensor_tensor(newM[:], b_mean[:], dmns[:],
                                    AluOpType.subtract)
            nc.vector.tensor_scalar(newM[:], newM[:], g01[:, 0:1], None,
                                    AluOpType.mult)
            nc.vector.tensor_tensor(newM[:], dmns[:], newM[:], AluOpType.add)
            nc.sync.dma_start(o_means.ap(), newM[:])
            newV = sg.tile([ND, D], f32)
            nc.vector.tensor_tensor(newV[:], b_var[:], dvrs[:],
                                    AluOpType.subtract)
            nc.vector.tensor_scalar(newV[:], newV[:], g01[:, 0:1], None,
                                    AluOpType.mult)
            nc.vector.tensor_tensor(newV[:], dvrs[:], newV[:], AluOpType.add)
            nc.sync.dma_start(o_vars.ap(), newV[:])

            def _colmean6(src_ap, nm):
                pt = pfin.tile([1, D], f32, tag="fp", name=f"pt_{nm}")
                nc.tensor.matmul(pt[:], ones6[:], src_ap, start=True,
                                 stop=True)
                out = sg.tile([1, D], f32, tag=nm, name=nm)
                nc.vector.tensor_scalar(out[:], pt[:], 1.0 / ND, None,
                                        AluOpType.mult)
                return out

            gm = _colmean6(newM[:], "gm")
            gv = _colmean6(newV[:], "gv")

            def _spread_loss(x_ap, g_ap, nm):
                x2 = wp.tile([ND, D], f32, tag="g1", name=f"x2_{nm}")
                nc.scalar.activation(x2[:], x_ap,
                                     mybir.ActivationFunctionType.Square)
                m2 = _colmean6(x2[:], f"m2_{nm}")
                g2 = wp.tile([1, D], f32, tag="g2", name=f"g2_{nm}")
                nc.scalar.activation(g2[:], g_ap,
                                     mybir.ActivationFunctionType.Square)
                df = wp.tile([1, D], f32, tag="g4", name=f"df_{nm}")
                nc.vector.tensor_tensor(df[:], m2[:], g2[:],
                                        AluOpType.subtract)
                out = sg.tile([1, 1], f32, tag=nm, name=nm)
                nc.vector.reduce_sum(out[:], df[:], axis=mybir.AxisListType.X)
                nc.vector.tensor_scalar(out[:], out[:], 1.0 / D, None,
                                        AluOpType.mult)
                return out

            l_mean = _spread_loss(newM[:], gm[:], "lmean")
            l_var = _spread_loss(newV[:], gv[:], "lvar")

            # mu_mean / mu_var from global sums
            mmp = pfin.tile([1, W], f32, tag="fp", name="mmp")
            nc.tensor.matmul(mmp[:], ones6[:], dsts[:], start=True, stop=True)
            mu_mean = sg.tile([1, D], f32)
            nc.vector.tensor_scalar(mu_mean[:], mmp[:, 0:D], 1.0 / B, None,
                                    AluOpType.mult)
            mu_sq = sg.tile([1, D], f32)
            nc.vector.tensor_scalar(mu_sq[:], mmp[:, D:W], 1.0 / B, None,
                                    AluOpType.mult)
            mm2 = wp.tile([1, D], f32, tag="g2", name="mm2")
            nc.scalar.activation(mm2[:], mu_mean[:],
                                 mybir.ActivationFunctionType.Square)
            mu_var = sg.tile([1, D], f32)
            nc.vector.tensor_tensor(mu_var[:], mu_sq[:], mm2[:],
                                    AluOpType.subtract)

            def _mse_row(a_ap, b_ap, nm):
                df = wp.tile([1, D], f32, tag="g2", name=f"df_{nm}")
                nc.vector.tensor_tensor(df[:], a_ap, b_ap,
                                        AluOpType.subtract)
                s2 = wp.tile([1, D], f32, tag="g3", name=f"s2_{nm}")
                out = sg.tile([1, 1], f32, tag=nm, name=nm)
                nc.scalar.activation(s2[:], df[:],
                                     mybir.ActivationFunctionType.Square,
                                     accum_out=out[:])
                nc.vector.tensor_scalar(out[:], out[:], 1.0 / D, None,
                                        AluOpType.mult)
                return out

            l_mu_mean = _mse_row(mu_mean[:], gm[:], "lmumean")
            l_mu_var = _mse_row(mu_var[:], gv[:], "lmuvar")

            lossp = pfin.tile([1, 1], f32, tag="fp", name="lossp")
            nc.tensor.matmul(lossp[:], ones128[:], li[:],
                             start=True, stop=False)
            nc.tensor.matmul(lossp[:], ones128[:], ri[:],
                             start=False, stop=False)
            one1 = sg.tile([1, 1], f32)
            nc.vector.memset(one1[:], 1.0)
            pieces = [l_mean, l_var, l_mu_mean, l_mu_var]
            for i, pc in enumerate(pieces):
                nc.tensor.matmul(lossp[:], one1[:], pc[:],
                                 start=False, stop=(i == len(pieces) - 1))
            lout = sg.tile([1, 1], f32)
            nc.vector.tensor_copy(lout[:], lossp[:])
            nc.sync.dma_start(o_loss.ap(), lout[:])
            pfinctx.__exit__(None, None, None)

    nc.compile()
    return nc


def _prep_inputs(mu_tilde, anchors, domain_means, domain_vars, y_true,
                 d_true):
    mu_tilde = np.asarray(mu_tilde, dtype=np.float32)
    anchors = np.ascontiguousarray(np.asarray(anchors, dtype=np.float32))
    domain_means = np.ascontiguousarray(
        np.asarray(domain_means, dtype=np.float32))
    domain_vars = np.ascontiguousarray(
        np.asarray(domain_vars, dtype=np.float32))
    y = np.asarray(y_true).astype(np.int64)
    d = np.asarray(d_true).astype(np.int64)

    mu16 = mu_tilde.astype(np.float16)

    # index metadata: counts + domain-sorted group packing
    seg_cnt = np.bincount(d * C + y, minlength=ND * C).reshape(ND, C)
    cnts = np.ascontiguousarray(seg_cnt.T.astype(np.float32))      # (128, 6)
    dcnt = seg_cnt.sum(axis=1).astype(np.float32).reshape(ND, 1)

    order = np.argsort(d, kind="stable")
    dom_counts = np.bincount(d, minlength=ND)
    # single-domain groups of GR rows, padded with -1
    groups = []   # (domain, idx array of len GR)
    pos = 0
    for dom in range(ND):
        n = int(dom_counts[dom])
        idx = order[pos:pos + n]
        pos += n
        ng = (n + GR - 1) // GR
        padded = np.full(ng * GR, -1, dtype=np.int64)
        padded[:n] = idx
        for k in range(ng):
            groups.append((dom, padded[k * GR:(k + 1) * GR]))
    assert len(groups) <= NCORES * NG, len(groups)
    while len(groups) < NCORES * NG:
        groups.append((-1, np.full(GR, -1, dtype=np.int64)))

    in_maps = []
    for i in range(NCORES):
        gs = groups[i * NG:(i + 1) * NG]
        idxs = np.concatenate([g[1] for g in gs])
        valid = idxs >= 0
        muc = np.zeros((R, D), dtype=np.float16)
        muc[valid] = mu16[idxs[valid]]
        yv = np.full(R, 999.0, dtype=np.float32)
        yv[valid] = y[idxs[valid]]
        yTc = np.ascontiguousarray(yv.reshape(NTp, P).T)
        dgoc = np.zeros((NG, ND), dtype=np.float32)
        for gi, (dom, _) in enumerate(gs):
            if dom >= 0:
                dgoc[gi, dom] = 1.0
        dgoc = np.ascontiguousarray(
            np.broadcast_to(dgoc.reshape(1, NG * ND), (P, NG * ND)))
        in_maps.append({
            "mu": muc,
            "yT": yTc,
            "dgo": dgoc,
            "cnts": cnts,
            "dcnt": dcnt,
            "anchors": anchors,
            "dmeans": domain_means,
            "dvars": domain_vars,
        })
    return in_maps


def get_compiled():
    global _compiled
    if _compiled is None:
        _compiled = _build()
    return _compiled


def run(in_maps, **kw):
    nc = get_compiled()
    return run_bass_kernel_spmd(nc, in_maps, core_ids=list(range(NCORES)),
                                **kw)


def kernel(mu_tilde, anchors, domain_means, domain_vars, y_true, d_true):
    in_maps = _prep_inputs(mu_tilde, anchors, domain_means, domain_vars,
                           y_true, d_true)
    res = run(in_maps)
    r0 = res.results[0]
    return (
        r0["o_anch"].astype(np.float32),
        r0["o_means"].astype(np.float32),
        r0["o_vars"].astype(np.float32),
        np.float32(r0["o_loss"].reshape(())),
    )


# revision 9
# speedup vs baseline: 1.7847x; 1.3130x over previous
"""AnchorBankCAA fused segment-mean/EMA/loss kernel for 8 TRN2 NeuronCores.

Strategy (data-parallel over B, rows domain-sorted host-side):
  - host sorts rows by domain and packs them into single-domain groups of
    3072 rows (24 tiles of 128), padded with inert rows (mu=0, y=999);
    22 groups per core (67584 rows, +3.1% padding)
  - mu ships as fp16 with per-tile layout [mu | mu^2-slot]; ACT/GpSimd
    alternate computing the squares into the slot
  - per tile: ONE matmul — class one-hot (is_equal vs iota) as stationary,
    [mu | mu^2] (128, 512) moving — accumulating [feature sums | sqsums]
    per class into a ping-pong PSUM stage bank
  - per group: 6 masked-identity matmuls flush the stage into 6 per-domain
    PSUM accumulators (mask = host-provided group-domain one-hot)
  - AllReduce the (128, 6*512) f32 partials, then a replicated final phase
    (EMA + CAA/stats losses) computes the outputs; counts come from a host
    bincount (index metadata only)
"""
import sys

sys.path.insert(0, "/opt/trn_rl_repo")

import numpy as np
from concourse import bacc, mybir
from concourse.alu_op_type import AluOpType
from concourse.tile import TileContext
from concourse.bass_utils import run_bass_kernel_spmd

C = 128          # classes
ND = 6           # domains
D = 256          # feat dim
B = 524288
NCORES = 8
P = 128
GT = 24          # tiles per group
GR = GT * P      # rows per group (3072)
NG = 22          # groups per core
NTp = NG * GT    # tiles per core (528)
R = NTp * P      # padded rows per core (67584)
MOM = 0.9
W = 2 * D        # 512: [sums | sqsums] stage width
CCN = P * ND * W  # AllReduce payload floats (128*6*512)

f32 = mybir.dt.float32
f16 = mybir.dt.float16
RG = [list(range(NCORES))]

_compiled = None


def _build():
    nc = bacc.Bacc(num_devices=NCORES)

    mu = nc.dram_tensor("mu", (R, D), f16, kind="ExternalInput")
    yT = nc.dram_tensor("yT", (P, NTp), f32, kind="ExternalInput")
    mids_d = nc.dram_tensor("mids", (P, NG * ND * C), f16,
                            kind="ExternalInput")
    invc = nc.dram_tensor("invc", (P, ND * D), f32, kind="ExternalInput")
    has01 = nc.dram_tensor("has01", (P, ND * D), f32, kind="ExternalInput")
    cnts = nc.dram_tensor("cnts", (P, ND), f32, kind="ExternalInput")
    dcnt = nc.dram_tensor("dcnt", (ND, 1), f32, kind="ExternalInput")
    anchors = nc.dram_tensor("anchors", (ND, C, D), f32, kind="ExternalInput")
    dmeans = nc.dram_tensor("dmeans", (ND, D), f32, kind="ExternalInput")
    dvars = nc.dram_tensor("dvars", (ND, D), f32, kind="ExternalInput")

    o_anch = nc.dram_tensor("o_anch", (ND, C, D), f32, kind="ExternalOutput")
    o_means = nc.dram_tensor("o_means", (ND, D), f32, kind="ExternalOutput")
    o_vars = nc.dram_tensor("o_vars", (ND, D), f32, kind="ExternalOutput")
    o_loss = nc.dram_tensor("o_loss", (1, 1), f32, kind="ExternalOutput")

    dst_scr = nc.dram_tensor("dst_scr", (ND, W), f32, kind="Internal")
    bf16 = mybir.dt.bfloat16
    cc_in = nc.dram_tensor("cc_in", (CCN,), bf16, kind="Internal")
    cc_out = nc.dram_tensor("cc_out", (CCN,), bf16, kind="Internal",
                            addr_space="Shared")

    iota128_d = nc.inline_tensor(
        np.tile(np.arange(C, dtype=np.float16), (P, 1)), "iota128")
    ident16_d = nc.inline_tensor(np.eye(P, dtype=np.float16), "ident16")
    ident_d = nc.inline_tensor(np.eye(P, dtype=np.float32), "ident")
    offdiag_d = nc.inline_tensor(
        (1.0 - np.eye(C, dtype=np.float32)), "offdiag")

    with TileContext(nc) as tc:
        with (
            tc.tile_pool(name="singles", bufs=1) as sg,
            tc.tile_pool(name="grp", bufs=2) as grp,
            tc.tile_pool(name="work", bufs=4) as wp,
        ):
            iota128 = sg.tile([P, C], f16)
            nc.sync.dma_start(iota128[:], iota128_d[:])
            yTs = sg.tile([P, NTp], f32)
            nc.sync.dma_start(yTs[:], yT[:])
            mids_s = sg.tile([P, NG * ND * C], f16)
            nc.sync.dma_start(mids_s[:], mids_d[:])

            accctx = tc.tile_pool(name="acc", bufs=1, space="PSUM")
            pacc = accctx.__enter__()
            stage = [pacc.tile([P, W], f32, tag=f"stage{k}",
                               name=f"stage{k}") for k in range(2)]
            finals = [pacc.tile([P, W], f32, tag=f"fin{d}",
                                name=f"fin{d}") for d in range(ND)]

            # dram view: group g, tile-in-group u, partition p, feat f
            muv = mu.ap().rearrange("(g u p) f -> g p u f", u=GT, p=P)
            for g in range(NG):
                gt = grp.tile([P, GT * W], f16, name="gt", tag="gt")
                # mu -> first half of each tile slot
                nc.sync.dma_start(
                    gt[:].rearrange("p (u w) -> p u w", u=GT)[:, :, 0:D],
                    muv[g])
                stg = stage[g % 2]
                for u in range(GT):
                    t = g * GT + u
                    msl = gt[:, u * W:u * W + D]
                    sqs = gt[:, u * W + D:(u + 1) * W]
                    # squares split 3:2 over ACT / GpSimd
                    if u % 5 < 3:
                        nc.scalar.square(sqs, msl)
                    else:
                        nc.gpsimd.tensor_tensor(sqs, msl, msl,
                                                AluOpType.mult)
                    oh = wp.tile([P, C], f16, tag="oh", name="oh")
                    nc.vector.tensor_scalar(
                        oh[:], iota128[:], yTs[:, t:t + 1], None,
                        AluOpType.is_equal)
                    # sums-MM depends only on the DMA; sqsums-MM on squares
                    nc.tensor.matmul(stg[:, 0:D], oh[:], msl,
                                     start=(u == 0), stop=(u == GT - 1))
                    nc.tensor.matmul(stg[:, D:W], oh[:], sqs,
                                     start=(u == 0), stop=(u == GT - 1))
                # flush stage into per-domain finals via masked identities
                stgs = wp.tile([P, W], f16, tag="stgs", name="stgs")
                nc.vector.tensor_copy(stgs[:], stg[:])
                for d in range(ND):
                    k = (g * ND + d) * C
                    nc.tensor.matmul(finals[d][:], mids_s[:, k:k + C],
                                     stgs[:], start=(g == 0),
                                     stop=(g == NG - 1))

            # ---- pack partials (bf16) and AllReduce ----
            ccb = sg.tile([P, ND * W], mybir.dt.bfloat16)
            for d in range(ND):
                nc.vector.tensor_copy(ccb[:, d * W:(d + 1) * W],
                                      finals[d][:])
            accctx.__exit__(None, None, None)
            pfinctx = tc.tile_pool(name="pfin", bufs=4, space="PSUM")
            pfin = pfinctx.__enter__()

            nc.sync.dma_start(
                cc_in.ap().rearrange("(p j) -> p j", p=P), ccb[:])
            nc.gpsimd.collective_compute(
                "AllReduce", AluOpType.add, replica_groups=RG,
                ins=[cc_in.ap()], outs=[cc_out.ap()])
            segg_b = sg.tile([P, ND * W], mybir.dt.bfloat16)
            nc.sync.dma_start(
                segg_b[:], cc_out.ap().rearrange("(p j) -> p j", p=P))
            segg = sg.tile([P, ND * W], f32)
            nc.vector.tensor_copy(segg[:], segg_b[:])

            # ---- replicated final phase ----
            ident = sg.tile([P, P], f32)
            nc.sync.dma_start(ident[:], ident_d[:])
            offdiag = sg.tile([C, C], f32)
            nc.sync.dma_start(offdiag[:], offdiag_d[:])
            anch = sg.tile([P, ND * D], f32)
            nc.sync.dma_start(
                anch[:].rearrange("c (a f) -> c a f", a=ND),
                anchors.ap().rearrange("a c f -> c a f"))
            dmns = sg.tile([ND, D], f32)
            nc.sync.dma_start(dmns[:], dmeans.ap())
            dvrs = sg.tile([ND, D], f32)
            nc.sync.dma_start(dvrs[:], dvars.ap())
            cnts_s = sg.tile([P, ND], f32)
            nc.sync.dma_start(cnts_s[:], cnts.ap())
            dcnt_s = sg.tile([ND, 1], f32)
            nc.sync.dma_start(dcnt_s[:], dcnt.ap())
            invc_s = sg.tile([P, ND * D], f32)
            nc.sync.dma_start(invc_s[:], invc.ap())
            has01_s = sg.tile([P, ND * D], f32)
            nc.sync.dma_start(has01_s[:], has01.ap())

            ones128 = sg.tile([P, 1], f32)
            nc.vector.memset(ones128[:], 1.0)
            ones6 = sg.tile([ND, 1], f32)
            nc.vector.memset(ones6[:], 1.0)
            onesrow = sg.tile([1, C], f32)
            nc.vector.memset(onesrow[:], 1.0)

            # new anchors: A + 0.1*(cnt>0)*(seg_mean - A), vectorized
            segv = segg[:].rearrange("c (a w) -> c a w", a=ND)[:, :, 0:D]
            mean_a = sg.tile([P, ND * D], f32)
            nc.vector.tensor_tensor(
                mean_a[:].rearrange("c (a f) -> c a f", a=ND), segv,
                invc_s[:].rearrange("c (a f) -> c a f", a=ND),
                AluOpType.mult)
            diff_a = sg.tile([P, ND * D], f32)
            nc.vector.tensor_tensor(diff_a[:], mean_a[:], anch[:],
                                    AluOpType.subtract)
            nc.vector.tensor_tensor(diff_a[:], diff_a[:], has01_s[:],
                                    AluOpType.mult)
            newA = sg.tile([P, ND * D], f32)
            nc.vector.tensor_tensor(newA[:], anch[:], diff_a[:],
                                    AluOpType.add)
            nc.sync.dma_start(
                o_anch.ap().rearrange("a c f -> c a f"),
                newA[:].rearrange("c (a f) -> c a f", a=ND))

            # class mean over domains (= A_mean)
            cm = sg.tile([P, D], f32)
            nc.vector.tensor_tensor(cm[:], newA[:, 0:D], newA[:, D:2 * D],
                                    AluOpType.add)
            for d in range(2, ND):
                nc.vector.tensor_tensor(cm[:], cm[:],
                                        newA[:, d * D:(d + 1) * D],
                                        AluOpType.add)
            nc.vector.tensor_scalar(cm[:], cm[:], 1.0 / ND, None,
                                    AluOpType.mult)

            # loss_intra = mean((newA - cm)^2)
            li = sg.tile([P, 1], f32)
            sqscr = wp.tile([P, D], f32, tag="fD", name="sqscr")
            for d in range(ND):
                dif = wp.tile([P, D], f32, tag="fD2", name=f"dif{d}")
                nc.vector.tensor_tensor(dif[:], newA[:, d * D:(d + 1) * D],
                                        cm[:], AluOpType.subtract)
                ac = wp.tile([P, 1], f32, tag=f"li{d}", name=f"liac{d}")
                nc.scalar.activation(sqscr[:], dif[:],
                                     mybir.ActivationFunctionType.Square,
                                     accum_out=ac[:])
                if d == 0:
                    nc.vector.tensor_copy(li[:], ac[:])
                else:
                    nc.vector.tensor_tensor(li[:], li[:], ac[:],
                                            AluOpType.add)
            nc.vector.tensor_scalar(li[:], li[:], 1.0 / (ND * C * D), None,
                                    AluOpType.mult)

            # loss_inter: pairwise distances of cm rows
            sqp = sg.tile([P, 1], f32)
            cm2 = wp.tile([P, D], f32, tag="fD", name="cm2")
            nc.scalar.activation(cm2[:], cm[:],
                                 mybir.ActivationFunctionType.Square,
                                 accum_out=sqp[:])
            amt = sg.tile([P, D], f32)
            amtn = sg.tile([P, D], f32)
            for k in range(2):
                trp = pfin.tile([P, P], f32, tag="fp", name=f"trp{k}")
                nc.tensor.transpose(trp[:], cm[:, k * P:(k + 1) * P],
                                    ident[:])
                nc.vector.tensor_copy(amt[:, k * P:(k + 1) * P], trp[:])
                nc.vector.tensor_scalar(amtn[:, k * P:(k + 1) * P], trp[:],
                                        -2.0, None, AluOpType.mult)
            sqrp = pfin.tile([1, P], f32, tag="fp", name="sqrp")
            nc.tensor.transpose(sqrp[:], sqp[:], ident[:])
            sqr = sg.tile([1, C], f32)
            nc.vector.tensor_copy(sqr[:], sqrp[:])

            d2p = pfin.tile([P, C], f32, tag="fp", name="d2p")
            nc.tensor.matmul(d2p[:], amt[:, 0:P], amtn[:, 0:P],
                             start=True, stop=False)
            nc.tensor.matmul(d2p[:], amt[:, P:2 * P], amtn[:, P:2 * P],
                             start=False, stop=False)
            nc.tensor.matmul(d2p[:], onesrow[:], sqr[:],
                             start=False, stop=False)
            nc.tensor.matmul(d2p[:], sqr[:], onesrow[:],
                             start=False, stop=True)
            d2s = sg.tile([P, C], f32)
            nc.vector.tensor_scalar(d2s[:], d2p[:], 1e-12, None,
                                    AluOpType.max)
            dst = wp.tile([P, C], f32, tag="fD", name="dst")
            nc.scalar.activation(dst[:], d2s[:],
                                 mybir.ActivationFunctionType.Sqrt)
            rel = wp.tile([P, C], f32, tag="fD2", name="rel")
            nc.scalar.activation(rel[:], dst[:],
                                 mybir.ActivationFunctionType.Relu,
                                 bias=1.0, scale=-1.0)
            nc.vector.tensor_tensor(rel[:], rel[:], offdiag[:],
                                    AluOpType.mult)
            ri = sg.tile([P, 1], f32)
            nc.vector.reduce_sum(ri[:], rel[:], axis=mybir.AxisListType.X)
            nc.vector.tensor_scalar(ri[:], ri[:], 1.0 / (C * (C - 1)), None,
                                    AluOpType.mult)

            # per-domain stats: [d_sum | d_sq] = column sums over classes
            for d in range(ND):
                pt = pfin.tile([1, W], f32, tag="fp", name=f"pt{d}")
                nc.tensor.matmul(pt[:], ones128[:],
                                 segg[:, d * W:(d + 1) * W],
                                 start=True, stop=True)
                row = wp.tile([1, W], f32, tag="dstrow", name=f"row{d}")
                nc.vector.tensor_copy(row[:], pt[:])
                nc.sync.dma_start(dst_scr.ap()[d:d + 1, :], row[:])
            dsts = sg.tile([ND, W], f32)
            nc.sync.dma_start(dsts[:], dst_scr.ap())
            d_sum = dsts[:, 0:D]
            d_sq = dsts[:, D:W]

            safe = sg.tile([ND, 1], f32)
            nc.vector.tensor_scalar(safe[:], dcnt_s[:], 1.0, None,
                                    AluOpType.max)
            rec6 = sg.tile([ND, 1], f32)
            nc.vector.reciprocal(rec6[:], safe[:])
            b_mean = sg.tile([ND, D], f32)
            nc.vector.tensor_scalar(b_mean[:], d_sum, rec6[:, 0:1], None,
                                    AluOpType.mult)
            bm2 = wp.tile([ND, D], f32, tag="g1", name="bm2")
            nc.scalar.activation(bm2[:], b_mean[:],
                                 mybir.ActivationFunctionType.Square)
            nc.vector.tensor_scalar(bm2[:], bm2[:], safe[:, 0:1], None,
                                    AluOpType.mult)
            b_var = sg.tile([ND, D], f32)
            nc.vector.tensor_tensor(b_var[:], d_sq, bm2[:],
                                    AluOpType.subtract)
            cm1 = sg.tile([ND, 1], f32)
            nc.vector.tensor_scalar(cm1[:], dcnt_s[:], -1.0, 1.0,
                                    AluOpType.add, AluOpType.max)
            recd = sg.tile([ND, 1], f32)
            nc.vector.reciprocal(recd[:], cm1[:])
            nc.vector.tensor_scalar(b_var[:], b_var[:], recd[:, 0:1], None,
                                    AluOpType.mult)
            g01 = sg.tile([ND, 1], f32)
            nc.vector.tensor_scalar(g01[:], dcnt_s[:], 1.0, 1.0 - MOM,
                                    AluOpType.is_gt, AluOpType.mult)

            newM = sg.tile([ND, D], f32)
            nc.vector.tensor_tensor(newM[:], b_mean[:], dmns[:],
                                    AluOpType.subtract)
            nc.vector.tensor_scalar(newM[:], newM[:], g01[:, 0:1], None,
                                    AluOpType.mult)
            nc.vector.tensor_tensor(newM[:], dmns[:], newM[:], AluOpType.add)
            nc.sync.dma_start(o_means.ap(), newM[:])
            newV = sg.tile([ND, D], f32)
            nc.vector.tensor_tensor(newV[:], b_var[:], dvrs[:],
                                    AluOpType.subtract)
            nc.vector.tensor_scalar(newV[:], newV[:], g01[:, 0:1], None,
                                    AluOpType.mult)
            nc.vector.tensor_tensor(newV[:], dvrs[:], newV[:], AluOpType.add)
            nc.sync.dma_start(o_vars.ap(), newV[:])

            def _colmean6(src_ap, nm):
                pt = pfin.tile([1, D], f32, tag="fp", name=f"pt_{nm}")
                nc.tensor.matmul(pt[:], ones6[:], src_ap, start=True,
                                 stop=True)
                out = sg.tile([1, D], f32, tag=nm, name=nm)
                nc.vector.tensor_scalar(out[:], pt[:], 1.0 / ND, None,
                                        AluOpType.mult)
                return out

            gm = _colmean6(newM[:], "gm")
            gv = _colmean6(newV[:], "gv")

            def _spread_loss(x_ap, g_ap, nm):
                x2 = wp.tile([ND, D], f32, tag="g1", name=f"x2_{nm}")
                nc.scalar.activation(x2[:], x_ap,
                                     mybir.ActivationFunctionType.Square)
                m2 = _colmean6(x2[:], f"m2_{nm}")
                g2 = wp.tile([1, D], f32, tag="g2", name=f"g2_{nm}")
                nc.scalar.activation(g2[:], g_ap,
                                     mybir.ActivationFunctionType.Square)
                df = wp.tile([1, D], f32, tag="g4", name=f"df_{nm}")
                nc.vector.tensor_tensor(df[:], m2[:], g2[:],
                                        AluOpType.subtract)
                out = sg.tile([1, 1], f32, tag=nm, name=nm)
                nc.vector.reduce_sum(out[:], df[:], axis=mybir.AxisListType.X)
                nc.vector.tensor_scalar(out[:], out[:], 1.0 / D, None,
                                        AluOpType.mult)
                return out

            l_mean = _spread_loss(newM[:], gm[:], "lmean")
            l_var = _spread_loss(newV[:], gv[:], "lvar")

            # mu_mean / mu_var from global sums
            mmp = pfin.tile([1, W], f32, tag="fp", name="mmp")
            nc.tensor.matmul(mmp[:], ones6[:], dsts[:], start=True, stop=True)
            mu_mean = sg.tile([1, D], f32)
            nc.vector.tensor_scalar(mu_mean[:], mmp[:, 0:D], 1.0 / B, None,
                                    AluOpType.mult)
            mu_sq = sg.tile([1, D], f32)
            nc.vector.tensor_scalar(mu_sq[:], mmp[:, D:W], 1.0 / B, None,
                                    AluOpType.mult)
            mm2 = wp.tile([1, D], f32, tag="g2", name="mm2")
            nc.scalar.activation(mm2[:], mu_mean[:],
                                 mybir.ActivationFunctionType.Square)
            mu_var = sg.tile([1, D], f32)
            nc.vector.tensor_tensor(mu_var[:], mu_sq[:], mm2[:],
                                    AluOpType.subtract)

            def _mse_row(a_ap, b_ap, nm):
                df = wp.tile([1, D], f32, tag="g2", name=f"df_{nm}")
                nc.vector.tensor_tensor(df[:], a_ap, b_ap,
                                        AluOpType.subtract)
                s2 = wp.tile([1, D], f32, tag="g3", name=f"s2_{nm}")
                out = sg.tile([1, 1], f32, tag=nm, name=nm)
                nc.scalar.activation(s2[:], df[:],
                                     mybir.ActivationFunctionType.Square,
                                     accum_out=out[:])
                nc.vector.tensor_scalar(out[:], out[:], 1.0 / D, None,
                                        AluOpType.mult)
                return out

            l_mu_mean = _mse_row(mu_mean[:], gm[:], "lmumean")
            l_mu_var = _mse_row(mu_var[:], gv[:], "lmuvar")

            lossp = pfin.tile([1, 1], f32, tag="fp", name="lossp")
            nc.tensor.matmul(lossp[:], ones128[:], li[:],
                             start=True, stop=False)
            nc.tensor.matmul(lossp[:], ones128[:], ri[:],
                             start=False, stop=False)
            one1 = sg.tile([1, 1], f32)
            nc.vector.memset(one1[:], 1.0)
            pieces = [l_mean, l_var, l_mu_mean, l_mu_var]
            for i, pc in enumerate(pieces):
                nc.tensor.matmul(lossp[:], one1[:], pc[:],
                                 start=False, stop=(i == len(pieces) - 1))
            lout = sg.tile([1, 1], f32)
            nc.vector.tensor_copy(lout[:], lossp[:])
            nc.sync.dma_start(o_loss.ap(), lout[:])
            pfinctx.__exit__(None, None, None)

    nc.compile()
    return nc


def _prep_inputs(mu_tilde, anchors, domain_means, domain_vars, y_true,
                 d_true):
    mu_tilde = np.asarray(mu_tilde, dtype=np.float32)
    anchors = np.ascontiguousarray(np.asarray(anchors, dtype=np.float32))
    domain_means = np.ascontiguousarray(
        np.asarray(domain_means, dtype=np.float32))
    domain_vars = np.ascontiguousarray(
        np.asarray(domain_vars, dtype=np.float32))
    y = np.asarray(y_true).astype(np.int64)
    d = np.asarray(d_true).astype(np.int64)

    mu16 = mu_tilde.astype(np.float16)

    # index metadata: counts + domain-sorted group packing
    seg_cnt = np.bincount(d * C + y, minlength=ND * C).reshape(ND, C)
    cnts = np.ascontiguousarray(seg_cnt.T.astype(np.float32))      # (128, 6)
    dcnt = seg_cnt.sum(axis=1).astype(np.float32).reshape(ND, 1)
    # (128, 6*256) broadcast rows: 1/max(cnt,1) and 0.1*(cnt>0) per (c, d)
    invr = (1.0 / np.maximum(seg_cnt, 1)).astype(np.float32)       # (6, 128)
    invc_bc = np.ascontiguousarray(np.repeat(
        invr.T[:, :, None], D, axis=2).reshape(P, ND * D))
    hasr = ((seg_cnt > 0) * (1.0 - MOM)).astype(np.float32)
    has01_bc = np.ascontiguousarray(np.repeat(
        hasr.T[:, :, None], D, axis=2).reshape(P, ND * D))

    order = np.argsort(d, kind="stable")
    dom_counts = np.bincount(d, minlength=ND)
    # single-domain groups of GR rows, padded with -1
    groups = []   # (domain, idx array of len GR)
    pos = 0
    for dom in range(ND):
        n = int(dom_counts[dom])
        idx = order[pos:pos + n]
        pos += n
        ng = (n + GR - 1) // GR
        padded = np.full(ng * GR, -1, dtype=np.int64)
        padded[:n] = idx
        for k in range(ng):
            groups.append((dom, padded[k * GR:(k + 1) * GR]))
    assert len(groups) <= NCORES * NG, len(groups)
    while len(groups) < NCORES * NG:
        groups.append((-1, np.full(GR, -1, dtype=np.int64)))

    in_maps = []
    for i in range(NCORES):
        gs = groups[i * NG:(i + 1) * NG]
        idxs = np.concatenate([g[1] for g in gs])
        valid = idxs >= 0
        muc = np.zeros((R, D), dtype=np.float16)
        muc[valid] = mu16[idxs[valid]]
        yv = np.full(R, 999.0, dtype=np.float32)
        yv[valid] = y[idxs[valid]]
        yTc = np.ascontiguousarray(yv.reshape(NTp, P).T)
        mids = np.zeros((NG * ND, P, C), dtype=np.float16)
        eye = np.eye(P, dtype=np.float16)
        for gi, (dom, _) in enumerate(gs):
            if dom >= 0:
                mids[gi * ND + dom] = eye
        mids = np.ascontiguousarray(
            mids.transpose(1, 0, 2).reshape(P, NG * ND * C))
        in_maps.append({
            "mu": muc,
            "yT": yTc,
            "mids": mids,
            "invc": invc_bc,
            "has01": has01_bc,
            "cnts": cnts,
            "dcnt": dcnt,
            "anchors": anchors,
            "dmeans": domain_means,
            "dvars": domain_vars,
        })
    return in_maps


def get_compiled():
    global _compiled
    if _compiled is None:
        _compiled = _build()
    return _compiled


def run(in_maps, **kw):
    nc = get_compiled()
    return run_bass_kernel_spmd(nc, in_maps, core_ids=list(range(NCORES)),
                                **kw)


def kernel(mu_tilde, anchors, domain_means, domain_vars, y_true, d_true):
    in_maps = _prep_inputs(mu_tilde, anchors, domain_means, domain_vars,
                           y_true, d_true)
    res = run(in_maps)
    r0 = res.results[0]
    return (
        r0["o_anch"].astype(np.float32),
        r0["o_means"].astype(np.float32),
        r0["o_vars"].astype(np.float32),
        np.float32(r0["o_loss"].reshape(())),
    )


# revision 11
# speedup vs baseline: 1.9889x; 1.1144x over previous
"""AnchorBankCAA fused segment-mean/EMA/loss kernel for 8 TRN2 NeuronCores.

Strategy (data-parallel over B, rows domain-sorted host-side):
  - host sorts rows by domain and packs them into single-domain groups of
    3072 rows (24 tiles of 128), padded with inert rows (mu=0, y=999);
    22 groups per core (67584 rows, +3.1% padding)
  - mu ships as fp16 with per-tile layout [mu | mu^2-slot]; ACT/GpSimd
    alternate computing the squares into the slot
  - per tile: ONE matmul — class one-hot (is_equal vs iota) as stationary,
    [mu | mu^2] (128, 512) moving — accumulating [feature sums | sqsums]
    per class into a ping-pong PSUM stage bank
  - per group: 6 masked-identity matmuls flush the stage into 6 per-domain
    PSUM accumulators (mask = host-provided group-domain one-hot)
  - AllReduce the (128, 6*512) f32 partials, then a replicated final phase
    (EMA + CAA/stats losses) computes the outputs; counts come from a host
    bincount (index metadata only)
"""
import sys

sys.path.insert(0, "/opt/trn_rl_repo")

import numpy as np
from concourse import bacc, mybir
from concourse.alu_op_type import AluOpType
from concourse.tile import TileContext
from concourse.bass_utils import run_bass_kernel_spmd

C = 128          # classes
ND = 6           # domains
D = 256          # feat dim
B = 524288
NCORES = 8
P = 128
GT = 24          # tiles per group
GR = GT * P      # rows per group (3072)
NG = 22          # groups per core
NTp = NG * GT    # tiles per core (528)
R = NTp * P      # padded rows per core (67584)
MOM = 0.9
W = 2 * D        # 512: [sums | sqsums] stage width
CCN = P * ND * W  # AllReduce payload floats (128*6*512)

f32 = mybir.dt.float32
f16 = mybir.dt.float16
RG = [list(range(NCORES))]

_compiled = None


def _build():
    nc = bacc.Bacc(num_devices=NCORES)

    mu = nc.dram_tensor("mu", (R, D), f16, kind="ExternalInput")
    yT = nc.dram_tensor("yT", (P, NTp), f32, kind="ExternalInput")
    mids_d = nc.dram_tensor("mids", (P, NG * ND * C), f16,
                            kind="ExternalInput")
    invc = nc.dram_tensor("invc", (P, ND * D), f32, kind="ExternalInput")
    has01 = nc.dram_tensor("has01", (P, ND * D), f32, kind="ExternalInput")
    cnts = nc.dram_tensor("cnts", (P, ND), f32, kind="ExternalInput")
    dcnt = nc.dram_tensor("dcnt", (ND, 1), f32, kind="ExternalInput")
    anchors = nc.dram_tensor("anchors", (ND, C, D), f32, kind="ExternalInput")
    dmeans = nc.dram_tensor("dmeans", (ND, D), f32, kind="ExternalInput")
    dvars = nc.dram_tensor("dvars", (ND, D), f32, kind="ExternalInput")

    o_anch = nc.dram_tensor("o_anch", (ND, C, D), f32, kind="ExternalOutput")
    o_means = nc.dram_tensor("o_means", (ND, D), f32, kind="ExternalOutput")
    o_vars = nc.dram_tensor("o_vars", (ND, D), f32, kind="ExternalOutput")
    o_loss = nc.dram_tensor("o_loss", (1, 1), f32, kind="ExternalOutput")

    dst_scr = nc.dram_tensor("dst_scr", (ND, W), f32, kind="Internal")
    bf16 = mybir.dt.bfloat16
    cc_in = nc.dram_tensor("cc_in", (CCN,), bf16, kind="Internal")
    cc_out = nc.dram_tensor("cc_out", (CCN,), bf16, kind="Internal",
                            addr_space="Shared")

    iota128_d = nc.inline_tensor(
        np.tile(np.arange(C, dtype=np.float16), (P, 1)), "iota128")
    ident16_d = nc.inline_tensor(np.eye(P, dtype=np.float16), "ident16")
    ident_d = nc.inline_tensor(np.eye(P, dtype=np.float32), "ident")
    offdiag_d = nc.inline_tensor(
        (1.0 - np.eye(C, dtype=np.float32)), "offdiag")

    with TileContext(nc) as tc:
        with (
            tc.tile_pool(name="singles", bufs=1) as sg,
            tc.tile_pool(name="grp", bufs=3) as grp,
            tc.tile_pool(name="work", bufs=2) as wp,
        ):
            iota128 = sg.tile([P, C], f16)
            nc.sync.dma_start(iota128[:], iota128_d[:])
            yTs = sg.tile([P, NTp], f32)
            nc.sync.dma_start(yTs[:], yT[:])

            accctx = tc.tile_pool(name="acc", bufs=1, space="PSUM")
            pacc = accctx.__enter__()
            stage = [pacc.tile([P, W], f32, tag=f"stage{k}",
                               name=f"stage{k}") for k in range(2)]
            finals = [pacc.tile([P, W], f32, tag=f"fin{d}",
                                name=f"fin{d}") for d in range(ND)]

            # dram view: group g, tile-in-group u, partition p, feat f
            muv = mu.ap().rearrange("(g u p) f -> g p u f", u=GT, p=P)
            for g in range(NG):
                gt = grp.tile([P, GT * W], f16, name="gt", tag="gt")
                # mu -> first half of each tile slot
                nc.sync.dma_start(
                    gt[:].rearrange("p (u w) -> p u w", u=GT)[:, :, 0:D],
                    muv[g])
                mid_g = wp.tile([P, ND * C], f16, tag="midg", name="midg", bufs=3)
                nc.sync.dma_start(
                    mid_g[:], mids_d.ap()[:, g * ND * C:(g + 1) * ND * C])
                stg = stage[g % 2]
                for u in range(GT):
                    t = g * GT + u
                    msl = gt[:, u * W:u * W + D]
                    sqs = gt[:, u * W + D:(u + 1) * W]
                    # squares split 3:2 over ACT / GpSimd
                    if u % 5 < 3:
                        nc.scalar.square(sqs, msl)
                    else:
                        nc.gpsimd.tensor_tensor(sqs, msl, msl,
                                                AluOpType.mult)
                    oh = wp.tile([P, C], f16, tag="oh", name="oh", bufs=8)
                    nc.vector.tensor_scalar(
                        oh[:], iota128[:], yTs[:, t:t + 1], None,
                        AluOpType.is_equal)
                    # sums-MM depends only on the DMA; sqsums-MM on squares
                    nc.tensor.matmul(stg[:, 0:D], oh[:], msl,
                                     start=(u == 0), stop=(u == GT - 1))
                    nc.tensor.matmul(stg[:, D:W], oh[:], sqs,
                                     start=(u == 0), stop=(u == GT - 1))
                # flush stage into per-domain finals via masked identities
                stgs = wp.tile([P, W], f16, tag="stgs", name="stgs")
                nc.vector.tensor_copy(stgs[:], stg[:])
                for d in range(ND):
                    nc.tensor.matmul(finals[d][:], mid_g[:, d * C:(d + 1) * C],
                                     stgs[:], start=(g == 0),
                                     stop=(g == NG - 1))

            # ---- pack partials (bf16) and AllReduce ----
            ccb = sg.tile([P, ND * W], mybir.dt.bfloat16)
            for d in range(ND):
                nc.vector.tensor_copy(ccb[:, d * W:(d + 1) * W],
                                      finals[d][:])
            accctx.__exit__(None, None, None)
            pfinctx = tc.tile_pool(name="pfin", bufs=4, space="PSUM")
            pfin = pfinctx.__enter__()

            nc.sync.dma_start(
                cc_in.ap().rearrange("(p j) -> p j", p=P), ccb[:])
            nc.gpsimd.collective_compute(
                "AllReduce", AluOpType.add, replica_groups=RG,
                ins=[cc_in.ap()], outs=[cc_out.ap()])
            segg_b = sg.tile([P, ND * W], mybir.dt.bfloat16)
            nc.sync.dma_start(
                segg_b[:], cc_out.ap().rearrange("(p j) -> p j", p=P))
            segg = sg.tile([P, ND * W], f32)
            nc.vector.tensor_copy(segg[:], segg_b[:])

            # ---- replicated final phase ----
            ident = sg.tile([P, P], f32)
            nc.sync.dma_start(ident[:], ident_d[:])
            offdiag = sg.tile([C, C], f32)
            nc.sync.dma_start(offdiag[:], offdiag_d[:])
            anch = sg.tile([P, ND * D], f32)
            nc.sync.dma_start(
                anch[:].rearrange("c (a f) -> c a f", a=ND),
                anchors.ap().rearrange("a c f -> c a f"))
            dmns = sg.tile([ND, D], f32)
            nc.sync.dma_start(dmns[:], dmeans.ap())
            dvrs = sg.tile([ND, D], f32)
            nc.sync.dma_start(dvrs[:], dvars.ap())
            cnts_s = sg.tile([P, ND], f32)
            nc.sync.dma_start(cnts_s[:], cnts.ap())
            dcnt_s = sg.tile([ND, 1], f32)
            nc.sync.dma_start(dcnt_s[:], dcnt.ap())
            invc_s = sg.tile([P, ND * D], f32)
            nc.sync.dma_start(invc_s[:], invc.ap())
            has01_s = sg.tile([P, ND * D], f32)
            nc.sync.dma_start(has01_s[:], has01.ap())

            ones128 = sg.tile([P, 1], f32)
            nc.vector.memset(ones128[:], 1.0)
            ones6 = sg.tile([ND, 1], f32)
            nc.vector.memset(ones6[:], 1.0)
            onesrow = sg.tile([1, C], f32)
            nc.vector.memset(onesrow[:], 1.0)

            # new anchors: A + 0.1*(cnt>0)*(seg_mean - A), vectorized
            segv = segg[:].rearrange("c (a w) -> c a w", a=ND)[:, :, 0:D]
            mean_a = sg.tile([P, ND * D], f32)
            nc.vector.tensor_tensor(
                mean_a[:].rearrange("c (a f) -> c a f", a=ND), segv,
                invc_s[:].rearrange("c (a f) -> c a f", a=ND),
                AluOpType.mult)
            diff_a = sg.tile([P, ND * D], f32)
            nc.vector.tensor_tensor(diff_a[:], mean_a[:], anch[:],
                                    AluOpType.subtract)
            nc.vector.tensor_tensor(diff_a[:], diff_a[:], has01_s[:],
                                    AluOpType.mult)
            newA = sg.tile([P, ND * D], f32)
            nc.vector.tensor_tensor(newA[:], anch[:], diff_a[:],
                                    AluOpType.add)
            nc.sync.dma_start(
                o_anch.ap().rearrange("a c f -> c a f"),
                newA[:].rearrange("c (a f) -> c a f", a=ND))

            # class mean over domains (= A_mean): strided reduce
            cm = sg.tile([P, D], f32)
            nc.vector.reduce_sum(
                cm[:], newA[:].rearrange("c (a f) -> c f a", a=ND),
                axis=mybir.AxisListType.X)
            nc.vector.tensor_scalar(cm[:], cm[:], 1.0 / ND, None,
                                    AluOpType.mult)

            # loss_inter helper: sqp = row sums of cm^2 (also used for
            # loss_intra via the E[A^2] - cm^2 identity)
            sqp = sg.tile([P, 1], f32)
            cm2 = wp.tile([P, D], f32, tag="fD", name="cm2")
            nc.scalar.activation(cm2[:], cm[:],
                                 mybir.ActivationFunctionType.Square,
                                 accum_out=sqp[:])

            # loss_intra = [sum(newA^2) - 6*sum(cm^2)] / (6*128*256)
            liA = sg.tile([P, 1], f32)
            sqscr = wp.tile([P, ND * D], f32, tag="sqbig", name="sqscr", bufs=1)
            nc.scalar.activation(sqscr[:], newA[:],
                                 mybir.ActivationFunctionType.Square,
                                 accum_out=liA[:])
            li = sg.tile([P, 1], f32)
            nc.vector.tensor_scalar(li[:], sqp[:], -float(ND), None,
                                    AluOpType.mult)
            nc.vector.tensor_tensor(li[:], liA[:], li[:], AluOpType.add)
            nc.vector.tensor_scalar(li[:], li[:], 1.0 / (ND * C * D), None,
                                    AluOpType.mult)
            amt = sg.tile([P, D], f32)
            amtn = sg.tile([P, D], f32)
            for k in range(2):
                trp = pfin.tile([P, P], f32, tag="fp", name=f"trp{k}")
                nc.tensor.transpose(trp[:], cm[:, k * P:(k + 1) * P],
                                    ident[:])
                nc.vector.tensor_copy(amt[:, k * P:(k + 1) * P], trp[:])
                nc.vector.tensor_scalar(amtn[:, k * P:(k + 1) * P], trp[:],
                                        -2.0, None, AluOpType.mult)
            sqrp = pfin.tile([1, P], f32, tag="fp", name="sqrp")
            nc.tensor.transpose(sqrp[:], sqp[:], ident[:])
            sqr = sg.tile([1, C], f32)
            nc.vector.tensor_copy(sqr[:], sqrp[:])

            d2p = pfin.tile([P, C], f32, tag="fp", name="d2p")
            nc.tensor.matmul(d2p[:], amt[:, 0:P], amtn[:, 0:P],
                             start=True, stop=False)
            nc.tensor.matmul(d2p[:], amt[:, P:2 * P], amtn[:, P:2 * P],
                             start=False, stop=False)
            nc.tensor.matmul(d2p[:], onesrow[:], sqr[:],
                             start=False, stop=False)
            nc.tensor.matmul(d2p[:], sqr[:], onesrow[:],
                             start=False, stop=True)
            d2s = sg.tile([P, C], f32)
            nc.vector.tensor_scalar(d2s[:], d2p[:], 1e-12, None,
                                    AluOpType.max)
            dst = wp.tile([P, C], f32, tag="fD", name="dst")
            nc.scalar.activation(dst[:], d2s[:],
                                 mybir.ActivationFunctionType.Sqrt)
            rel = wp.tile([P, C], f32, tag="fD2", name="rel")
            nc.scalar.activation(rel[:], dst[:],
                                 mybir.ActivationFunctionType.Relu,
                                 bias=1.0, scale=-1.0)
            nc.vector.tensor_tensor(rel[:], rel[:], offdiag[:],
                                    AluOpType.mult)
            ri = sg.tile([P, 1], f32)
            nc.vector.reduce_sum(ri[:], rel[:], axis=mybir.AxisListType.X)
            nc.vector.tensor_scalar(ri[:], ri[:], 1.0 / (C * (C - 1)), None,
                                    AluOpType.mult)

            # per-domain stats: [d_sum | d_sq] = column sums over classes
            rowall = sg.tile([1, ND * W], f32)
            for d in range(ND):
                pt = pfin.tile([1, W], f32, tag="fp", name=f"pt{d}")
                nc.tensor.matmul(pt[:], ones128[:],
                                 segg[:, d * W:(d + 1) * W],
                                 start=True, stop=True)
                nc.vector.tensor_copy(rowall[:, d * W:(d + 1) * W], pt[:])
            nc.sync.dma_start(
                dst_scr.ap().rearrange("a w -> (a w)")[None, :], rowall[:])
            dsts = sg.tile([ND, W], f32)
            nc.sync.dma_start(dsts[:], dst_scr.ap())
            d_sum = dsts[:, 0:D]
            d_sq = dsts[:, D:W]

            safe = sg.tile([ND, 1], f32)
            nc.vector.tensor_scalar(safe[:], dcnt_s[:], 1.0, None,
                                    AluOpType.max)
            rec6 = sg.tile([ND, 1], f32)
            nc.vector.reciprocal(rec6[:], safe[:])
            b_mean = sg.tile([ND, D], f32)
            nc.vector.tensor_scalar(b_mean[:], d_sum, rec6[:, 0:1], None,
                                    AluOpType.mult)
            bm2 = wp.tile([ND, D], f32, tag="g1", name="bm2")
            nc.scalar.activation(bm2[:], b_mean[:],
                                 mybir.ActivationFunctionType.Square)
            nc.vector.tensor_scalar(bm2[:], bm2[:], safe[:, 0:1], None,
                                    AluOpType.mult)
            b_var = sg.tile([ND, D], f32)
            nc.vector.tensor_tensor(b_var[:], d_sq, bm2[:],
                                    AluOpType.subtract)
            cm1 = sg.tile([ND, 1], f32)
            nc.vector.tensor_scalar(cm1[:], dcnt_s[:], -1.0, 1.0,
                                    AluOpType.add, AluOpType.max)
            recd = sg.tile([ND, 1], f32)
            nc.vector.reciprocal(recd[:], cm1[:])
            nc.vector.tensor_scalar(b_var[:], b_var[:], recd[:, 0:1], None,
                                    AluOpType.mult)
            g01 = sg.tile([ND, 1], f32)
            nc.vector.tensor_scalar(g01[:], dcnt_s[:], 1.0, 1.0 - MOM,
                                    AluOpType.is_gt, AluOpType.mult)

            newM = sg.tile([ND, D], f32)
            nc.vector.tensor_tensor(newM[:], b_mean[:], dmns[:],
                                    AluOpType.subtract)
            nc.vector.tensor_scalar(newM[:], newM[:], g01[:, 0:1], None,
                                    AluOpType.mult)
            nc.vector.tensor_tensor(newM[:], dmns[:], newM[:], AluOpType.add)
            nc.sync.dma_start(o_means.ap(), newM[:])
            newV = sg.tile([ND, D], f32)
            nc.vector.tensor_tensor(newV[:], b_var[:], dvrs[:],
                                    AluOpType.subtract)
            nc.vector.tensor_scalar(newV[:], newV[:], g01[:, 0:1], None,
                                    AluOpType.mult)
            nc.vector.tensor_tensor(newV[:], dvrs[:], newV[:], AluOpType.add)
            nc.sync.dma_start(o_vars.ap(), newV[:])

            def _colmean6(src_ap, nm):
                pt = pfin.tile([1, D], f32, tag="fp", name=f"pt_{nm}")
                nc.tensor.matmul(pt[:], ones6[:], src_ap, start=True,
                                 stop=True)
                out = sg.tile([1, D], f32, tag=nm, name=nm)
                nc.vector.tensor_scalar(out[:], pt[:], 1.0 / ND, None,
                                        AluOpType.mult)
                return out

            gm = _colmean6(newM[:], "gm")
            gv = _colmean6(newV[:], "gv")

            def _spread_loss(x_ap, g_ap, nm):
                x2 = wp.tile([ND, D], f32, tag="g1", name=f"x2_{nm}")
                nc.scalar.activation(x2[:], x_ap,
                                     mybir.ActivationFunctionType.Square)
                m2 = _colmean6(x2[:], f"m2_{nm}")
                g2 = wp.tile([1, D], f32, tag="g2", name=f"g2_{nm}")
                nc.scalar.activation(g2[:], g_ap,
                                     mybir.ActivationFunctionType.Square)
                df = wp.tile([1, D], f32, tag="g4", name=f"df_{nm}")
                nc.vector.tensor_tensor(df[:], m2[:], g2[:],
                                        AluOpType.subtract)
                out = sg.tile([1, 1], f32, tag=nm, name=nm)
                nc.vector.reduce_sum(out[:], df[:], axis=mybir.AxisListType.X)
                nc.vector.tensor_scalar(out[:], out[:], 1.0 / D, None,
                                        AluOpType.mult)
                return out

            l_mean = _spread_loss(newM[:], gm[:], "lmean")
            l_var = _spread_loss(newV[:], gv[:], "lvar")

            # mu_mean / mu_var from global sums
            mmp = pfin.tile([1, W], f32, tag="fp", name="mmp")
            nc.tensor.matmul(mmp[:], ones6[:], dsts[:], start=True, stop=True)
            mu_mean = sg.tile([1, D], f32)
            nc.vector.tensor_scalar(mu_mean[:], mmp[:, 0:D], 1.0 / B, None,
                                    AluOpType.mult)
            mu_sq = sg.tile([1, D], f32)
            nc.vector.tensor_scalar(mu_sq[:], mmp[:, D:W], 1.0 / B, None,
                                    AluOpType.mult)
            mm2 = wp.tile([1, D], f32, tag="g2", name="mm2")
            nc.scalar.activation(mm2[:], mu_mean[:],
                                 mybir.ActivationFunctionType.Square)
            mu_var = sg.tile([1, D], f32)
            nc.vector.tensor_tensor(mu_var[:], mu_sq[:], mm2[:],
                                    AluOpType.subtract)

            def _mse_row(a_ap, b_ap, nm):
                df = wp.tile([1, D], f32, tag="g2", name=f"df_{nm}")
                nc.vector.tensor_tensor(df[:], a_ap, b_ap,
                                        AluOpType.subtract)
                s2 = wp.tile([1, D], f32, tag="g3", name=f"s2_{nm}")
                out = sg.tile([1, 1], f32, tag=nm, name=nm)
                nc.scalar.activation(s2[:], df[:],
                                     mybir.ActivationFunctionType.Square,
                                     accum_out=out[:])
                nc.vector.tensor_scalar(out[:], out[:], 1.0 / D, None,
                                        AluOpType.mult)
                return out

            l_mu_mean = _mse_row(mu_mean[:], gm[:], "lmumean")
            l_mu_var = _mse_row(mu_var[:], gv[:], "lmuvar")

            lossp = pfin.tile([1, 1], f32, tag="fp", name="lossp")
            nc.tensor.matmul(lossp[:], ones128[:], li[:],
                             start=True, stop=False)
            nc.tensor.matmul(lossp[:], ones128[:], ri[:],
                             start=False, stop=False)
            one1 = sg.tile([1, 1], f32)
            nc.vector.memset(one1[:], 1.0)
            pieces = [l_mean, l_var, l_mu_mean, l_mu_var]
            for i, pc in enumerate(pieces):
                nc.tensor.matmul(lossp[:], one1[:], pc[:],
                                 start=False, stop=(i == len(pieces) - 1))
            lout = sg.tile([1, 1], f32)
            nc.vector.tensor_copy(lout[:], lossp[:])
            nc.sync.dma_start(o_loss.ap(), lout[:])
            pfinctx.__exit__(None, None, None)

    nc.compile()
    return nc


def _prep_inputs(mu_tilde, anchors, domain_means, domain_vars, y_true,
                 d_true):
    mu_tilde = np.asarray(mu_tilde, dtype=np.float32)
    anchors = np.ascontiguousarray(np.asarray(anchors, dtype=np.float32))
    domain_means = np.ascontiguousarray(
        np.asarray(domain_means, dtype=np.float32))
    domain_vars = np.ascontiguousarray(
        np.asarray(domain_vars, dtype=np.float32))
    y = np.asarray(y_true).astype(np.int64)
    d = np.asarray(d_true).astype(np.int64)

    mu16 = mu_tilde.astype(np.float16)

    # index metadata: counts + domain-sorted group packing
    seg_cnt = np.bincount(d * C + y, minlength=ND * C).reshape(ND, C)
    cnts = np.ascontiguousarray(seg_cnt.T.astype(np.float32))      # (128, 6)
    dcnt = seg_cnt.sum(axis=1).astype(np.float32).reshape(ND, 1)
    # (128, 6*256) broadcast rows: 1/max(cnt,1) and 0.1*(cnt>0) per (c, d)
    invr = (1.0 / np.maximum(seg_cnt, 1)).astype(np.float32)       # (6, 128)
    invc_bc = np.ascontiguousarray(np.repeat(
        invr.T[:, :, None], D, axis=2).reshape(P, ND * D))
    hasr = ((seg_cnt > 0) * (1.0 - MOM)).astype(np.float32)
    has01_bc = np.ascontiguousarray(np.repeat(
        hasr.T[:, :, None], D, axis=2).reshape(P, ND * D))

    order = np.argsort(d, kind="stable")
    dom_counts = np.bincount(d, minlength=ND)
    # single-domain groups of GR rows, padded with -1
    groups = []   # (domain, idx array of len GR)
    pos = 0
    for dom in range(ND):
        n = int(dom_counts[dom])
        idx = order[pos:pos + n]
        pos += n
        ng = (n + GR - 1) // GR
        padded = np.full(ng * GR, -1, dtype=np.int64)
        padded[:n] = idx
        for k in range(ng):
            groups.append((dom, padded[k * GR:(k + 1) * GR]))
    assert len(groups) <= NCORES * NG, len(groups)
    while len(groups) < NCORES * NG:
        groups.append((-1, np.full(GR, -1, dtype=np.int64)))

    in_maps = []
    for i in range(NCORES):
        gs = groups[i * NG:(i + 1) * NG]
        idxs = np.concatenate([g[1] for g in gs])
        valid = idxs >= 0
        muc = np.zeros((R, D), dtype=np.float16)
        muc[valid] = mu16[idxs[valid]]
        yv = np.full(R, 999.0, dtype=np.float32)
        yv[valid] = y[idxs[valid]]
        yTc = np.ascontiguousarray(yv.reshape(NTp, P).T)
        mids = np.zeros((NG * ND, P, C), dtype=np.float16)
        eye = np.eye(P, dtype=np.float16)
        for gi, (dom, _) in enumerate(gs):
            if dom >= 0:
                mids[gi * ND + dom] = eye
        mids = np.ascontiguousarray(
            mids.transpose(1, 0, 2).reshape(P, NG * ND * C))
        in_maps.append({
            "mu": muc,
            "yT": yTc,
            "mids": mids,
            "invc": invc_bc,
            "has01": has01_bc,
            "cnts": cnts,
            "dcnt": dcnt,
            "anchors": anchors,
            "dmeans": domain_means,
            "dvars": domain_vars,
        })
    return in_maps


def get_compiled():
    global _compiled
    if _compiled is None:
        _compiled = _build()
    return _compiled


def run(in_maps, **kw):
    nc = get_compiled()
    return run_bass_kernel_spmd(nc, in_maps, core_ids=list(range(NCORES)),
                                **kw)


def kernel(mu_tilde, anchors, domain_means, domain_vars, y_true, d_true):
    in_maps = _prep_inputs(mu_tilde, anchors, domain_means, domain_vars,
                           y_true, d_true)
    res = run(in_maps)
    r0 = res.results[0]
    return (
        r0["o_anch"].astype(np.float32),
        r0["o_means"].astype(np.float32),
        r0["o_vars"].astype(np.float32),
        np.float32(r0["o_loss"].reshape(())),
    )
